# revision 15
# baseline (speedup 1.0000x reference)
"""Trainium2 Bass kernel for BioGNN (3-layer GAT + BN + global mean pool + MLP).

Distribution (8 NeuronCores, SPMD):
  - Nodes padded to NV = 8*T*128, sharded contiguously; core c owns T
    dst-tiles of 128 nodes.
  - Edges (incl. self-loops) partitioned by owner(dst), sorted by dst;
    per (core, dst-tile) edge lists are padded to a per-tile-uniform
    multiple of 128 (max over cores) so all cores run one program. Pad
    edges point at an all-zero table row and dst_local=999 so they
    contribute exactly zero.
  - Per dst-tile: dma_gather pulls per-edge rows [hp | a_src] (by src)
    and [a_dst] (by dst) from replicated HBM tables; ex =
    exp(leakyrelu(a_s+a_d)); a 0/1 selector T (iota==dst_local, built on
    VectorE) scaled by ex gives S_h; TensorE matmuls accumulate segment
    sums (messages) and softmax denominators in PSUM. Max-subtraction is
    skipped (it cancels exactly; logits are O(1) so fp32 exp is safe).
  - BN: per-core stats via thin matmuls (pads are exactly zero), 4KB
    AllReduce, applied fused with ReLU on ScalarE in transposed layout;
    next layer's table hp = relu(bn(z)) @ W^T computed per shard,
    AllGathered (table + small a_s/a_d table).
  - Mean pool via per-tile batch-selector matmul + tiny AllReduce; MLP
    head replicated.
"""
import numpy as np
from contextlib import ExitStack

import concourse.bass as bass
import concourse.tile as tile
from concourse import bacc, mybir
from concourse.bass_utils import run_bass_kernel_spmd
from concourse.masks import make_identity

P = 128
F32 = mybir.dt.float32
I16 = mybir.dt.int16
NCORES = 8
HID = 128
H = 4
D = H * HID            # 512
BIGROW = D + 64        # 576 f32 = 2304B rows for table2
BIGROW3 = HID + 64     # 192 f32 = 768B rows for table3
SMROW = 64             # 256B rows for small tables
CLS = 5
G = 50
EPS = 1e-5


class Cfg:
    def __init__(self, N, E, nch):
        self.N = N
        self.E = E
        self.nch = list(nch)
        self.T = len(nch)
        self.NV = NCORES * self.T * P
        self.s_off = np.cumsum([0] + [c * 8 for c in nch]).tolist()
        self.c_off = np.cumsum([0] + list(nch)).tolist()
        self.sumS = int(self.s_off[-1])
        self.sumC = int(self.c_off[-1])

    def key(self):
        return (self.N, self.E, tuple(self.nch))


# ----------------------------------------------------------------------------
# device program
# ----------------------------------------------------------------------------

def build_program(cfg):
    nc = bacc.Bacc("TRN2", target_bir_lowering=False, debug=False,
                   num_devices=NCORES)
    T, NV = cfg.T, cfg.NV
    rg = [list(range(NCORES))]

    def di(name, shape, dtype=F32):
        return nc.dram_tensor(name, shape, dtype, kind="ExternalInput")

    small1_d = di("small1", [NV, SMROW])
    idx_src_d = di("idx_src", [P, cfg.sumS], I16)
    idx_dst_d = di("idx_dst", [P, cfg.sumS], I16)
    dstloc_d = di("dstloc", [P, cfg.sumC])
    iota_d = di("iota", [P, P])
    w1rows_d = di("w1rows", [H, D])
    w2pre_d = di("w2pre", [P, 4, D])
    b2_d = di("b2c", [P, 4, 8])
    w3pre_d = di("w3pre", [P, 4, HID])
    b3c_d = di("b3c", [P, 4, 2])
    gbe1_d = di("gbe1", [P, 8])
    gbe2_d = di("gbe2", [P, 8])
    b3bc_d = di("b3bc", [P, HID])
    ptile_d = di("ptile", [P, T, G])
    rcnt_d = di("rcnt", [G, 1])
    mask_d = di("maskt", [P, T])
    lw1T_d = di("lw1T", [HID, HID // 2])
    lb1_d = di("lb1r", [1, HID // 2])
    lw2T_d = di("lw2T", [HID // 2, CLS])
    lb2_d = di("lb2r", [1, CLS])

    out_d = nc.dram_tensor("out", [G, CLS], F32, kind="ExternalOutput")

    with ExitStack() as stk:
        tc = stk.enter_context(tile.TileContext(nc))
        sbc = stk.enter_context(tc.tile_pool(name="const", bufs=1))
        wsm = stk.enter_context(tc.tile_pool(name="wsm", bufs=2))
        dram = stk.enter_context(tc.tile_pool(name="dram", bufs=1, space="DRAM"))

        # ---- persistent SBUF constants
        def load(name, dr, shape, dtype=F32):
            t = sbc.tile(shape, dtype, tag=name)
            nc.sync.dma_start(t[:], dr[:])
            return t

        idx_src = load("idx_src", idx_src_d, [P, cfg.sumS], I16)
        idx_dst = load("idx_dst", idx_dst_d, [P, cfg.sumS], I16)
        dstloc = load("dstloc", dstloc_d, [P, cfg.sumC])
        iota = load("iota", iota_d, [P, P])
        w1rows = load("w1rows", w1rows_d, [H, D])
        w2pre = load("w2pre", w2pre_d, [P, 4, D])
        b2c = load("b2c", b2_d, [P, 4, 8])
        w3pre = load("w3pre", w3pre_d, [P, 4, HID])
        b3c = load("b3c", b3c_d, [P, 4, 2])
        gbe1 = load("gbe1", gbe1_d, [P, 8])
        gbe2 = load("gbe2", gbe2_d, [P, 8])
        b3bc = load("b3bc", b3bc_d, [P, HID])
        ptile = load("ptile", ptile_d, [P, T, G])
        rcnt = load("rcnt", rcnt_d, [G, 1])
        maskt = load("maskt", mask_d, [P, T])
        lw1T = load("lw1T", lw1T_d, [HID, HID // 2])
        lb1r = load("lb1r", lb1_d, [1, HID // 2])
        lw2T = load("lw2T", lw2T_d, [HID // 2, CLS])
        lb2r = load("lb2r", lb2_d, [1, CLS])

        ident = sbc.tile([P, P], F32, tag="ident")
        make_identity(nc, ident[:])
        ones_col = sbc.tile([P, 1], F32, tag="ones_col")
        nc.vector.memset(ones_col[:], 1.0)
        ones_row = sbc.tile([1, 64], F32, tag="ones_row")
        nc.vector.memset(ones_row[:], 1.0)
        eps_col = sbc.tile([P, 1], F32, tag="eps_col")
        nc.vector.memset(eps_col[:], EPS)

        # ---- internal DRAM
        zdr = dram.tile([T, P, D], F32)
        ag2_in = dram.tile([T * P, BIGROW], F32)
        table2 = dram.tile([NV, BIGROW], F32, addr_space="Shared")
        ags2_in = dram.tile([T * P, SMROW], F32)
        small2 = dram.tile([NV, SMROW], F32, addr_space="Shared")
        ag3_in = dram.tile([T * P, BIGROW3], F32)
        table3 = dram.tile([NV, BIGROW3], F32, addr_space="Shared")
        ags3_in = dram.tile([T * P, SMROW], F32)
        small3 = dram.tile([NV, SMROW], F32, addr_space="Shared")
        st_io = [(dram.tile([P, 8], F32, name=f"st_in{i}"),
                  dram.tile([P, 8], F32, name=f"st_out{i}",
                            addr_space="Shared"))
                 for i in range(2)]
        pool_in = dram.tile([G, HID], F32)
        pool_out = dram.tile([G, HID], F32, addr_space="Shared")

        # ------------------------------------------------------------------
        def edge_ex(work, t, as_ap, ad_ap, nh):
            nch = cfg.nch[t]
            u = work.tile([P, nch, nh], F32, tag="u")
            nc.vector.tensor_tensor(out=u[:], in0=as_ap, in1=ad_ap,
                                    op=mybir.AluOpType.add)
            u2 = work.tile([P, nch, nh], F32, tag="u2")
            nc.vector.tensor_scalar(out=u2[:], in0=u[:], scalar1=0.2,
                                    scalar2=None, op0=mybir.AluOpType.mult)
            lr = work.tile([P, nch, nh], F32, tag="lr")
            nc.vector.tensor_tensor(out=lr[:], in0=u[:], in1=u2[:],
                                    op=mybir.AluOpType.max)
            ex = work.tile([P, nch, nh], F32, tag="ex")
            nc.scalar.activation(ex[:], lr[:], mybir.ActivationFunctionType.Exp)
            return ex

        def build_T(work, t):
            nch = cfg.nch[t]
            c0 = cfg.c_off[t]
            T_ = work.tile([P, nch, P], F32, tag="Tsel")
            nc.vector.tensor_tensor(
                out=T_[:],
                in0=iota[:].unsqueeze(1).to_broadcast([P, nch, P]),
                in1=dstloc[:, c0:c0 + nch].unsqueeze(2).to_broadcast([P, nch, P]),
                op=mybir.AluOpType.is_equal)
            return T_

        def recip_den(work, den_ap, nh):
            den_c = work.tile([P, nh], F32, tag="den_c")
            nc.vector.tensor_scalar(out=den_c[:], in0=den_ap, scalar1=1e-30,
                                    scalar2=None, op0=mybir.AluOpType.max)
            recip = work.tile([P, nh], F32, tag="recip")
            nc.vector.reciprocal(recip[:], den_c[:])
            return recip

        def tail_stats_zT(work, psum, t, z_sb, stats_acc):
            z2 = work.tile([P, D], F32, tag="z2")
            nc.scalar.activation(z2[:], z_sb[:],
                                 mybir.ActivationFunctionType.Square)
            stq = psum.tile([P, 8], F32, space="PSUM", tag="stq")
            for b in range(4):
                nc.tensor.matmul(out=stq[:, b:b + 1],
                                 lhsT=z_sb[:, b * P:(b + 1) * P],
                                 rhs=ones_col[:], start=True, stop=True)
                nc.tensor.matmul(out=stq[:, 4 + b:5 + b],
                                 lhsT=z2[:, b * P:(b + 1) * P],
                                 rhs=ones_col[:], start=True, stop=True)
            nc.vector.tensor_tensor(out=stats_acc[:], in0=stats_acc[:],
                                    in1=stq[:], op=mybir.AluOpType.add)
            zT = work.tile([P, D], F32, tag="zT")
            for b in range(4):
                ztp = psum.tile([P, P], F32, space="PSUM", tag="ztp")
                nc.tensor.transpose(out=ztp[:], in_=z_sb[:, b * P:(b + 1) * P],
                                    identity=ident[:])
                nc.vector.tensor_copy(out=zT[:, b * P:(b + 1) * P], in_=ztp[:])
            nc.sync.dma_start(zdr[t], zT[:])

        def phase_a_tile(work, psum, layer, t, stats_ps, pool_ps):
            nch = cfg.nch[t]
            NE = nch * P
            s0 = cfg.s_off[t]
            ssl = slice(s0, s0 + NE // 16)
            nh = H if layer < 3 else 1

            if layer == 1:
                Gs = work.tile([P, nch, SMROW], F32, tag="Gs")
                nc.gpsimd.dma_gather(Gs[:], small1_d[:], idx_src[:, ssl],
                                     NE, NE, SMROW, single_packet=False)
                Gd = work.tile([P, nch, SMROW], F32, tag="Gd")
                nc.gpsimd.dma_gather(Gd[:], small1_d[:], idx_dst[:, ssl],
                                     NE, NE, SMROW, single_packet=False)
                as_ap = Gs[:, :, 0:4]
                ad_ap = Gd[:, :, 4:8]
            else:
                tab = table2 if layer == 2 else table3
                sml = small2 if layer == 2 else small3
                brow = BIGROW if layer == 2 else BIGROW3
                dd = D if layer == 2 else HID
                Gb = work.tile([P, nch, brow], F32, tag="Gbig")
                nc.gpsimd.dma_gather(Gb[:], tab[:], idx_src[:, ssl],
                                     NE, NE, brow, single_packet=False)
                Gd = work.tile([P, nch, SMROW], F32, tag="Gd")
                nc.gpsimd.dma_gather(Gd[:], sml[:], idx_dst[:, ssl],
                                     NE, NE, SMROW, single_packet=False)
                as_ap = Gb[:, :, dd:dd + nh]
                ad_ap = Gd[:, :, 4:4 + nh]

            ex = edge_ex(work, t, as_ap, ad_ap, nh)
            T_ = build_T(work, t)

            if layer == 1:
                exd = work.tile([P, nch, H], F32, tag="exd")
                nc.vector.tensor_tensor(
                    out=exd[:], in0=ex[:],
                    in1=Gs[:, :, 8:9].to_broadcast([P, nch, H]),
                    op=mybir.AluOpType.mult)
                qd_ps = psum.tile([P, 8], F32, space="PSUM", tag="den")
                for c in range(nch):
                    nc.tensor.matmul(out=qd_ps[:, 0:4], lhsT=T_[:, c, :],
                                     rhs=ex[:, c, :], start=(c == 0),
                                     stop=(c == nch - 1))
                for c in range(nch):
                    nc.tensor.matmul(out=qd_ps[:, 4:8], lhsT=T_[:, c, :],
                                     rhs=exd[:, c, :], start=(c == 0),
                                     stop=(c == nch - 1))
                recip = recip_den(work, qd_ps[:, 0:4], H)
                t4 = work.tile([P, H], F32, tag="t4")
                nc.vector.tensor_tensor(out=t4[:], in0=qd_ps[:, 4:8],
                                        in1=recip[:], op=mybir.AluOpType.mult)
                tT_ps = psum.tile([H, P], F32, space="PSUM", tag="ztp")
                nc.tensor.transpose(out=tT_ps[:], in_=t4[:], identity=ident[:])
                tT = work.tile([H, P], F32, tag="tTs")
                nc.vector.tensor_copy(out=tT[:], in_=tT_ps[:])
                z_ps = psum.tile([P, D], F32, space="PSUM", tag="msg")
                nc.tensor.matmul(out=z_ps[:], lhsT=tT[:], rhs=w1rows[:],
                                 start=True, stop=True)
                z_sb = work.tile([P, D], F32, tag="z")
                nc.vector.tensor_copy(out=z_sb[:], in_=z_ps[:])
                tail_stats_zT(work, psum, t, z_sb, stats_ps)
            elif layer == 2:
                msg_ps = psum.tile([P, D], F32, space="PSUM", tag="msg")
                den_ps = psum.tile([P, H], F32, space="PSUM", tag="den")
                for h in range(H):
                    Sh = work.tile([P, nch, P], F32, tag="Sh")
                    nc.vector.tensor_tensor(
                        out=Sh[:], in0=T_[:],
                        in1=ex[:, :, h:h + 1].to_broadcast([P, nch, P]),
                        op=mybir.AluOpType.mult)
                    for c in range(nch):
                        nc.tensor.matmul(
                            out=msg_ps[:, h * P:(h + 1) * P], lhsT=Sh[:, c, :],
                            rhs=Gb[:, c, h * P:(h + 1) * P],
                            start=(c == 0), stop=(c == nch - 1))
                for c in range(nch):
                    nc.tensor.matmul(out=den_ps[:], lhsT=T_[:, c, :],
                                     rhs=ex[:, c, :], start=(c == 0),
                                     stop=(c == nch - 1))
                recip = recip_den(work, den_ps[:], H)
                z_sb = work.tile([P, D], F32, tag="z")
                for h in range(H):
                    nc.vector.tensor_scalar(
                        out=z_sb[:, h * P:(h + 1) * P],
                        in0=msg_ps[:, h * P:(h + 1) * P],
                        scalar1=recip[:, h:h + 1], scalar2=None,
                        op0=mybir.AluOpType.mult)
                tail_stats_zT(work, psum, t, z_sb, stats_ps)
            else:
                msg_ps = psum.tile([P, HID], F32, space="PSUM", tag="msg")
                den_ps = psum.tile([P, H], F32, space="PSUM", tag="den")
                Sh = work.tile([P, nch, P], F32, tag="Sh")
                nc.vector.tensor_tensor(
                    out=Sh[:], in0=T_[:],
                    in1=ex[:, :, 0:1].to_broadcast([P, nch, P]),
                    op=mybir.AluOpType.mult)
                for c in range(nch):
                    nc.tensor.matmul(out=msg_ps[:], lhsT=Sh[:, c, :],
                                     rhs=Gb[:, c, 0:HID],
                                     start=(c == 0), stop=(c == nch - 1))
                for c in range(nch):
                    nc.tensor.matmul(out=den_ps[:, 0:1], lhsT=T_[:, c, :],
                                     rhs=ex[:, c, :], start=(c == 0),
                                     stop=(c == nch - 1))
                recip = recip_den(work, den_ps[:, 0:1], 1)
                z_sb = work.tile([P, HID], F32, tag="z3")
                nc.vector.tensor_scalar(out=z_sb[:], in0=msg_ps[:],
                                        scalar1=recip[:, 0:1], scalar2=None,
                                        op0=mybir.AluOpType.mult)
                h3 = work.tile([P, HID], F32, tag="h3")
                nc.vector.tensor_tensor(out=h3[:], in0=z_sb[:], in1=b3bc[:],
                                        op=mybir.AluOpType.add)
                h3r = work.tile([P, HID], F32, tag="h3r")
                nc.scalar.activation(h3r[:], h3[:],
                                     mybir.ActivationFunctionType.Relu)
                pq = psum.tile([G, HID], F32, space="PSUM", tag="pq")
                nc.tensor.matmul(out=pq[:], lhsT=ptile[:, t, :],
                                 rhs=h3r[:], start=True, stop=True)
                nc.vector.tensor_tensor(out=pool_ps[:], in0=pool_ps[:],
                                        in1=pq[:], op=mybir.AluOpType.add)

        def finish_stats(stats_acc, gbe, sio):
            sin, sout = sio
            nc.sync.dma_start(sin[:], stats_acc[:])
            nc.gpsimd.collective_compute(
                "AllReduce", mybir.AluOpType.add, replica_groups=rg,
                ins=[sin[:].opt()], outs=[sout[:].opt()])
            stg = wsm.tile([P, 8], F32, tag="stg")
            nc.sync.dma_start(stg[:], sout[:])
            inv = 1.0 / cfg.N
            mean = wsm.tile([P, 4], F32, tag="bn_mean")
            nc.vector.tensor_scalar(out=mean[:], in0=stg[:, 0:4], scalar1=inv,
                                    scalar2=None, op0=mybir.AluOpType.mult)
            var = wsm.tile([P, 4], F32, tag="bn_var")
            nc.vector.tensor_scalar(out=var[:], in0=stg[:, 4:8], scalar1=inv,
                                    scalar2=None, op0=mybir.AluOpType.mult)
            mu2 = wsm.tile([P, 4], F32, tag="bn_mu2")
            nc.vector.tensor_tensor(out=mu2[:], in0=mean[:], in1=mean[:],
                                    op=mybir.AluOpType.mult)
            nc.vector.tensor_tensor(out=var[:], in0=var[:], in1=mu2[:],
                                    op=mybir.AluOpType.subtract)
            sd = wsm.tile([P, 4], F32, tag="bn_sd")
            nc.scalar.activation(sd[:], var[:],
                                 mybir.ActivationFunctionType.Sqrt,
                                 bias=eps_col[:])
            rcp = wsm.tile([P, 4], F32, tag="bn_rcp")
            nc.vector.reciprocal(rcp[:], sd[:])
            scale = wsm.tile([P, 4], F32, tag="bn_scale")
            nc.vector.tensor_tensor(out=scale[:], in0=gbe[:, 0:4], in1=rcp[:],
                                    op=mybir.AluOpType.mult)
            msc = wsm.tile([P, 4], F32, tag="bn_msc")
            nc.vector.tensor_tensor(out=msc[:], in0=mean[:], in1=scale[:],
                                    op=mybir.AluOpType.mult)
            shift = wsm.tile([P, 4], F32, tag="bn_shift")
            nc.vector.tensor_tensor(out=shift[:], in0=gbe[:, 4:8], in1=msc[:],
                                    op=mybir.AluOpType.subtract)
            return scale, shift

        def phase_b_tile(work, psum, t, wpre, bcol, d_next, n_as, brow,
                         agb, ags, scale, shift):
            zT = work.tile([P, D], F32, tag="zTb")
            nc.sync.dma_start(zT[:], zdr[t])
            hbT = work.tile([P, D], F32, tag="hbT")
            for b in range(4):
                nc.scalar.activation(hbT[:, b * P:(b + 1) * P],
                                     zT[:, b * P:(b + 1) * P],
                                     mybir.ActivationFunctionType.Relu,
                                     bias=shift[:, b:b + 1],
                                     scale=scale[:, b:b + 1])
            hp_ps = psum.tile([P, d_next], F32, space="PSUM", tag="hp")
            ab_ps = psum.tile([P, 8], F32, space="PSUM", tag="ab")
            for b in range(4):
                nc.tensor.matmul(out=hp_ps[:], lhsT=hbT[:, b * P:(b + 1) * P],
                                 rhs=wpre[:, b, :], start=(b == 0),
                                 stop=(b == 3))
            for b in range(4):
                nc.tensor.matmul(out=ab_ps[:, 0:2 * n_as],
                                 lhsT=hbT[:, b * P:(b + 1) * P],
                                 rhs=bcol[:, b, :], start=(b == 0),
                                 stop=(b == 3))
            stage = work.tile([P, brow], F32, tag="stage")
            nc.vector.memset(stage[:, d_next + n_as:brow], 0.0)
            nc.vector.tensor_scalar(out=stage[:, 0:d_next], in0=hp_ps[:],
                                    scalar1=maskt[:, t:t + 1], scalar2=None,
                                    op0=mybir.AluOpType.mult)
            nc.vector.tensor_scalar(out=stage[:, d_next:d_next + n_as],
                                    in0=ab_ps[:, 0:n_as],
                                    scalar1=maskt[:, t:t + 1], scalar2=None,
                                    op0=mybir.AluOpType.mult)
            sms = work.tile([P, SMROW], F32, tag="sms")
            nc.vector.memset(sms[:], 0.0)
            nc.vector.tensor_scalar(out=sms[:, 4:4 + n_as],
                                    in0=ab_ps[:, n_as:2 * n_as],
                                    scalar1=maskt[:, t:t + 1], scalar2=None,
                                    op0=mybir.AluOpType.mult)
            nc.sync.dma_start(agb[t * P:(t + 1) * P, :], stage[:])
            nc.sync.dma_start(ags[t * P:(t + 1) * P, :], sms[:])

        def run_phase_a(layer):
            with ExitStack() as ps:
                work = ps.enter_context(
                    tc.tile_pool(name=f"wA{layer}", bufs=2))
                psum = ps.enter_context(
                    tc.tile_pool(name=f"pA{layer}", bufs=2, space="PSUM"))
                if layer < 3:
                    stats_acc = work.tile([P, 8], F32, tag="stats_acc")
                    nc.vector.memset(stats_acc[:], 0.0)
                    for t in range(T):
                        phase_a_tile(work, psum, layer, t, stats_acc, None)
                    gbe = gbe1 if layer == 1 else gbe2
                    return finish_stats(stats_acc, gbe, st_io[layer - 1])
                else:
                    pool_acc = wsm.tile([G, HID], F32, tag="pool_acc")
                    nc.vector.memset(pool_acc[:], 0.0)
                    for t in range(T):
                        phase_a_tile(work, psum, layer, t, None, pool_acc)
                    pool_sb = wsm.tile([G, HID], F32, tag="pool_sb")
                    nc.vector.tensor_scalar(out=pool_sb[:], in0=pool_acc[:],
                                            scalar1=rcnt[:], scalar2=None,
                                            op0=mybir.AluOpType.mult)
                    nc.sync.dma_start(pool_in[:], pool_sb[:])
                    return None

        def run_phase_b(layer, scale, shift):
            wpre = w2pre if layer == 1 else w3pre
            bcol = b2c if layer == 1 else b3c
            d_next = D if layer == 1 else HID
            n_as = 4 if layer == 1 else 1
            brow = BIGROW if layer == 1 else BIGROW3
            agb = ag2_in if layer == 1 else ag3_in
            ags = ags2_in if layer == 1 else ags3_in
            tab = table2 if layer == 1 else table3
            sml = small2 if layer == 1 else small3
            with ExitStack() as ps:
                work = ps.enter_context(
                    tc.tile_pool(name=f"wB{layer}", bufs=2))
                psum = ps.enter_context(
                    tc.tile_pool(name=f"pB{layer}", bufs=2, space="PSUM"))
                for t in range(T):
                    phase_b_tile(work, psum, t, wpre, bcol, d_next, n_as,
                                 brow, agb, ags, scale, shift)
            nc.gpsimd.collective_compute(
                "AllGather", mybir.AluOpType.bypass, replica_groups=rg,
                ins=[agb[:].opt()], outs=[tab[:].opt()])
            nc.gpsimd.collective_compute(
                "AllGather", mybir.AluOpType.bypass, replica_groups=rg,
                ins=[ags[:].opt()], outs=[sml[:].opt()])

        # ================== program ==================
        scale1, shift1 = run_phase_a(1)
        run_phase_b(1, scale1, shift1)
        scale2, shift2 = run_phase_a(2)
        run_phase_b(2, scale2, shift2)
        run_phase_a(3)

        nc.gpsimd.collective_compute(
            "AllReduce", mybir.AluOpType.add, replica_groups=rg,
            ins=[pool_in[:].opt()], outs=[pool_out[:].opt()])

        with ExitStack() as ps:
            psum = ps.enter_context(
                tc.tile_pool(name="pMLP", bufs=1, space="PSUM"))
            poolg = wsm.tile([G, HID], F32, tag="poolg")
            nc.sync.dma_start(poolg[:], pool_out[:])
            pT_ps = psum.tile([HID, G], F32, space="PSUM", tag="pT")
            nc.tensor.transpose(out=pT_ps[:], in_=poolg[:],
                                identity=ident[:G, :G])
            pT = wsm.tile([HID, G], F32, tag="pTs")
            nc.vector.tensor_copy(out=pT[:], in_=pT_ps[:])
            m1_ps = psum.tile([G, HID // 2], F32, space="PSUM", tag="m1")
            nc.tensor.matmul(out=m1_ps[:], lhsT=pT[:], rhs=lw1T[:],
                             start=True, stop=False)
            nc.tensor.matmul(out=m1_ps[:], lhsT=ones_row[:, 0:G], rhs=lb1r[:],
                             start=False, stop=True)
            m1 = wsm.tile([G, HID // 2], F32, tag="m1s")
            nc.scalar.activation(m1[:], m1_ps[:],
                                 mybir.ActivationFunctionType.Relu)
            m1T_ps = psum.tile([HID // 2, G], F32, space="PSUM", tag="m1T")
            nc.tensor.transpose(out=m1T_ps[:], in_=m1[:],
                                identity=ident[:G, :G])
            m1T = wsm.tile([HID // 2, G], F32, tag="m1Ts")
            nc.vector.tensor_copy(out=m1T[:], in_=m1T_ps[:])
            o_ps = psum.tile([G, CLS], F32, space="PSUM", tag="o")
            nc.tensor.matmul(out=o_ps[:], lhsT=m1T[:], rhs=lw2T[:],
                             start=True, stop=False)
            nc.tensor.matmul(out=o_ps[:], lhsT=ones_row[:, 0:G], rhs=lb2r[:],
                             start=False, stop=True)
            o_sb = wsm.tile([G, CLS], F32, tag="o_sb")
            nc.vector.tensor_copy(out=o_sb[:], in_=o_ps[:])
            nc.sync.dma_start(out_d[:], o_sb[:])

    return nc


# ----------------------------------------------------------------------------
# host-side preparation
# ----------------------------------------------------------------------------

def wrap_idx(vals, S):
    n = len(vals)
    a = np.zeros((16, S), np.int16)
    ii = np.arange(n)
    a[ii % 16, ii // 16] = vals.astype(np.int16)
    return np.tile(a, (8, 1))


def make_cfg_and_inputs(inputs):
    x = np.asarray(inputs["x"], np.float32).reshape(-1)
    ei = np.asarray(inputs["edge_index"]).astype(np.int64)
    batch = np.asarray(inputs["batch"]).astype(np.int64)
    N = x.shape[0]
    T = int(np.ceil(N / (NCORES * P)))
    NV = NCORES * T * P
    pernode = T * P

    loop = np.arange(N, dtype=np.int64)
    src = np.concatenate([ei[0], loop])
    dst = np.concatenate([ei[1], loop])
    E = src.shape[0]

    order = np.argsort(dst, kind="stable")
    src_s = src[order]
    dst_s = dst[order]
    owner = dst_s // pernode
    tile_id = (dst_s % pernode) // P

    counts = np.zeros((NCORES, T), np.int64)
    np.add.at(counts, (owner, tile_id), 1)
    ne_t = np.maximum(counts.max(axis=0), 1)
    ne_t = ((ne_t + P - 1) // P * P).astype(np.int64)
    nch = (ne_t // P).astype(np.int64)

    cfg = Cfg(N, E, nch.tolist())
    PAD = NV - 1

    idx_src_all, idx_dst_all, dstloc_all = [], [], []
    for c in range(NCORES):
        isrc = np.zeros((P, cfg.sumS), np.int16)
        idst = np.zeros((P, cfg.sumS), np.int16)
        dloc = np.full((P, cfg.sumC), 999.0, np.float32)
        base = c * pernode
        for t in range(T):
            lo = np.searchsorted(dst_s, base + t * P)
            hi = np.searchsorted(dst_s, base + (t + 1) * P)
            n = hi - lo
            NE = int(ne_t[t])
            sv = np.full(NE, PAD, np.int64)
            dv = np.full(NE, PAD, np.int64)
            dl = np.full(NE, 999.0, np.float32)
            sv[:n] = src_s[lo:hi]
            dv[:n] = dst_s[lo:hi]
            dl[:n] = (dst_s[lo:hi] - base - t * P).astype(np.float32)
            s0 = cfg.s_off[t]
            isrc[:, s0:s0 + NE // 16] = wrap_idx(sv, NE // 16)
            idst[:, s0:s0 + NE // 16] = wrap_idx(dv, NE // 16)
            c0 = cfg.c_off[t]
            dloc[:, c0:c0 + int(nch[t])] = dl.reshape(int(nch[t]), P).T
        idx_src_all.append(isrc)
        idx_dst_all.append(idst)
        dstloc_all.append(dloc)

    W1 = np.asarray(inputs["W1"], np.float32)
    as1 = np.asarray(inputs["as1"], np.float32)
    ad1 = np.asarray(inputs["ad1"], np.float32)
    W2 = np.asarray(inputs["W2"], np.float32)
    as2 = np.asarray(inputs["as2"], np.float32)
    ad2 = np.asarray(inputs["ad2"], np.float32)
    W3 = np.asarray(inputs["W3"], np.float32)
    as3 = np.asarray(inputs["as3"], np.float32)
    ad3 = np.asarray(inputs["ad3"], np.float32)
    g1 = np.asarray(inputs["g1"], np.float32)
    be1 = np.asarray(inputs["be1"], np.float32)
    g2 = np.asarray(inputs["g2"], np.float32)
    be2 = np.asarray(inputs["be2"], np.float32)
    b3 = np.asarray(inputs["b3"], np.float32)
    lw1 = np.asarray(inputs["lw1"], np.float32)
    lb1 = np.asarray(inputs["lb1"], np.float32)
    lw2 = np.asarray(inputs["lw2"], np.float32)
    lb2 = np.asarray(inputs["lb2"], np.float32)

    w1col = W1[:, 0]
    s1 = (w1col.reshape(H, HID) * as1).sum(1)
    d1 = (w1col.reshape(H, HID) * ad1).sum(1)

    small1 = np.zeros((NV, SMROW), np.float32)
    small1[:N, 0:4] = x[:, None] * s1[None, :]
    small1[:N, 4:8] = x[:, None] * d1[None, :]
    small1[:N, 8] = x

    w1rows = np.zeros((H, D), np.float32)
    for h in range(H):
        w1rows[h, h * HID:(h + 1) * HID] = w1col[h * HID:(h + 1) * HID]

    W2T = np.ascontiguousarray(W2.T)
    As2 = np.zeros((D, H), np.float32)
    Ad2 = np.zeros((D, H), np.float32)
    for h in range(H):
        As2[h * HID:(h + 1) * HID, h] = as2[h]
        Ad2[h * HID:(h + 1) * HID, h] = ad2[h]
    Bs2 = W2T @ As2
    Bd2 = W2T @ Ad2
    w2pre = np.ascontiguousarray(W2T.reshape(4, P, D).transpose(1, 0, 2))
    b2c = np.ascontiguousarray(
        np.concatenate([Bs2, Bd2], 1).reshape(4, P, 8).transpose(1, 0, 2))

    W3T = np.ascontiguousarray(W3.T)
    Bs3 = W3T @ as3.T
    Bd3 = W3T @ ad3.T
    w3pre = np.ascontiguousarray(W3T.reshape(4, P, HID).transpose(1, 0, 2))
    b3c = np.ascontiguousarray(
        np.concatenate([Bs3, Bd3], 1).reshape(4, P, 2).transpose(1, 0, 2))

    gbe1 = np.concatenate([g1.reshape(4, P).T, be1.reshape(4, P).T], 1)
    gbe2 = np.concatenate([g2.reshape(4, P).T, be2.reshape(4, P).T], 1)
    b3bc = np.tile(b3[None, :], (P, 1)).astype(np.float32)

    cnt = np.bincount(batch, minlength=G).astype(np.float32)
    rcnt = (1.0 / np.maximum(cnt, 1.0)).reshape(G, 1).astype(np.float32)

    ptile_all, mask_all = [], []
    for c in range(NCORES):
        pt = np.zeros((P, T, G), np.float32)
        mk = np.zeros((P, T), np.float32)
        base = c * pernode
        for t in range(T):
            ids = base + t * P + np.arange(P)
            real = ids < N
            mk[real, t] = 1.0
            bb = batch[ids[real]]
            pt[np.arange(P)[real], t, bb] = 1.0
        ptile_all.append(pt)
        mask_all.append(mk)

    iota = np.tile(np.arange(P, dtype=np.float32)[None, :], (P, 1))

    common = dict(
        small1=small1,
        iota=np.ascontiguousarray(iota, np.float32),
        w1rows=w1rows,
        w2pre=w2pre.astype(np.float32), b2c=b2c.astype(np.float32),
        w3pre=w3pre.astype(np.float32), b3c=b3c.astype(np.float32),
        gbe1=gbe1.astype(np.float32), gbe2=gbe2.astype(np.float32),
        b3bc=b3bc,
        rcnt=rcnt,
        lw1T=np.ascontiguousarray(lw1.T, np.float32),
        lb1r=lb1.reshape(1, -1).astype(np.float32),
        lw2T=np.ascontiguousarray(lw2.T, np.float32),
        lb2r=lb2.reshape(1, -1).astype(np.float32),
    )
    in_maps = []
    for c in range(NCORES):
        m = dict(common)
        m["idx_src"] = idx_src_all[c]
        m["idx_dst"] = idx_dst_all[c]
        m["dstloc"] = dstloc_all[c]
        m["ptile"] = ptile_all[c]
        m["maskt"] = mask_all[c]
        in_maps.append(m)
    return cfg, in_maps


# ----------------------------------------------------------------------------
# entry point
# ----------------------------------------------------------------------------

_CACHE = {}


def _get_program(cfg):
    key = cfg.key()
    if key not in _CACHE:
        nc = build_program(cfg)
        nc.compile()
        _CACHE[key] = nc
    return _CACHE[key]


def kernel(**inputs):
    cfg, in_maps = make_cfg_and_inputs(inputs)
    nc = _get_program(cfg)
    res = run_bass_kernel_spmd(nc, in_maps, core_ids=list(range(NCORES)))
    return np.asarray(res.results[0]["out"])


# revision 19
# speedup vs baseline: 21.9548x; 21.9548x over previous
"""Trainium2 Bass kernel for BioGNN (3-layer GAT + BN + global mean pool + MLP).

Distribution (8 NeuronCores, SPMD):
  - Nodes padded to NV = 8*T*128, sharded contiguously; core c owns T
    dst-tiles of 128 nodes.
  - Edges (incl. self-loops) partitioned by owner(dst), sorted by dst;
    per (core, dst-tile) edge lists are padded to a per-tile-uniform
    multiple of 128 (max over cores) so all cores run one program. Pad
    edges point at an all-zero table row and dst_local=999 so they
    contribute exactly zero.
  - Per dst-tile: dma_gather pulls per-edge rows [hp | a_src] (by src)
    and [a_dst] (by dst) from replicated HBM tables; ex =
    exp(leakyrelu(a_s+a_d)); a 0/1 selector T (iota==dst_local, built on
    VectorE) scaled by ex gives S_h; TensorE matmuls accumulate segment
    sums (messages) and softmax denominators in PSUM. Max-subtraction is
    skipped (it cancels exactly; logits are O(1) so fp32 exp is safe).
  - BN: per-core stats via thin matmuls (pads are exactly zero), 4KB
    AllReduce, applied fused with ReLU on ScalarE in transposed layout;
    next layer's table hp = relu(bn(z)) @ W^T computed per shard,
    AllGathered (table + small a_s/a_d table).
  - Mean pool via per-tile batch-selector matmul + tiny AllReduce; MLP
    head replicated.
"""
import numpy as np
from contextlib import ExitStack

import concourse.bass as bass
import concourse.tile as tile
from concourse import bacc, mybir
from concourse.bass_utils import run_bass_kernel_spmd
from concourse.masks import make_identity

P = 128
F32 = mybir.dt.float32
I16 = mybir.dt.int16
NCORES = 8
HID = 128
H = 4
D = H * HID            # 512
BIGROW = D + 64        # 576 f32 = 2304B rows for table2
BIGROW3 = HID + 64     # 192 f32 = 768B rows for table3
SMROW = 64             # 256B rows for small tables
CLS = 5
G = 50
EPS = 1e-5


class Cfg:
    def __init__(self, N, E, nch):
        self.N = N
        self.E = E
        self.nch = list(nch)
        self.T = len(nch)
        self.NV = NCORES * self.T * P
        self.s_off = np.cumsum([0] + [c * 8 for c in nch]).tolist()
        self.c_off = np.cumsum([0] + list(nch)).tolist()
        self.sumS = int(self.s_off[-1])
        self.sumC = int(self.c_off[-1])

    def key(self):
        return (self.N, self.E, tuple(self.nch))


# ----------------------------------------------------------------------------
# device program
# ----------------------------------------------------------------------------

def build_program(cfg, reps=1):
    nc = bacc.Bacc("TRN2", target_bir_lowering=False, debug=False,
                   num_devices=NCORES)
    T, NV = cfg.T, cfg.NV
    rg = [list(range(NCORES))]

    def di(name, shape, dtype=F32):
        return nc.dram_tensor(name, shape, dtype, kind="ExternalInput")

    small1_d = di("small1", [NV, SMROW])
    idx_src_d = di("idx_src", [P, cfg.sumS], I16)
    idx_dst_d = di("idx_dst", [P, cfg.sumS], I16)
    dstloc_d = di("dstloc", [P, cfg.sumC])
    iota_d = di("iota", [P, P])
    w1rows_d = di("w1rows", [H, D])
    w2pre_d = di("w2pre", [P, 4, D])
    b2_d = di("b2c", [P, 4, 8])
    w3pre_d = di("w3pre", [P, 4, HID])
    b3c_d = di("b3c", [P, 4, 2])
    gbe1_d = di("gbe1", [P, 8])
    gbe2_d = di("gbe2", [P, 8])
    b3bc_d = di("b3bc", [P, HID])
    ptile_d = di("ptile", [P, T, G])
    rcnt_d = di("rcnt", [G, 1])
    mask_d = di("maskt", [P, T])
    lw1T_d = di("lw1T", [HID, HID // 2])
    lb1_d = di("lb1r", [1, HID // 2])
    lw2T_d = di("lw2T", [HID // 2, CLS])
    lb2_d = di("lb2r", [1, CLS])

    out_d = nc.dram_tensor("out", [G, CLS], F32, kind="ExternalOutput")

    with ExitStack() as stk:
        tc = stk.enter_context(tile.TileContext(nc))
        sbc = stk.enter_context(tc.tile_pool(name="const", bufs=1))
        wsm = stk.enter_context(tc.tile_pool(name="wsm", bufs=2))
        dram = stk.enter_context(tc.tile_pool(name="dram", bufs=1, space="DRAM"))

        # ---- persistent SBUF constants
        def load(name, dr, shape, dtype=F32):
            t = sbc.tile(shape, dtype, tag=name)
            nc.sync.dma_start(t[:], dr[:])
            return t

        idx_src = load("idx_src", idx_src_d, [P, cfg.sumS], I16)
        idx_dst = load("idx_dst", idx_dst_d, [P, cfg.sumS], I16)
        dstloc = load("dstloc", dstloc_d, [P, cfg.sumC])
        iota = load("iota", iota_d, [P, P])
        w1rows = load("w1rows", w1rows_d, [H, D])
        w2pre = load("w2pre", w2pre_d, [P, 4, D])
        b2c = load("b2c", b2_d, [P, 4, 8])
        w3pre = load("w3pre", w3pre_d, [P, 4, HID])
        b3c = load("b3c", b3c_d, [P, 4, 2])
        gbe1 = load("gbe1", gbe1_d, [P, 8])
        gbe2 = load("gbe2", gbe2_d, [P, 8])
        b3bc = load("b3bc", b3bc_d, [P, HID])
        ptile = load("ptile", ptile_d, [P, T, G])
        rcnt = load("rcnt", rcnt_d, [G, 1])
        maskt = load("maskt", mask_d, [P, T])
        lw1T = load("lw1T", lw1T_d, [HID, HID // 2])
        lb1r = load("lb1r", lb1_d, [1, HID // 2])
        lw2T = load("lw2T", lw2T_d, [HID // 2, CLS])
        lb2r = load("lb2r", lb2_d, [1, CLS])

        ident = sbc.tile([P, P], F32, tag="ident")
        make_identity(nc, ident[:])
        ones_col = sbc.tile([P, 1], F32, tag="ones_col")
        nc.vector.memset(ones_col[:], 1.0)
        ones_row = sbc.tile([1, 64], F32, tag="ones_row")
        nc.vector.memset(ones_row[:], 1.0)
        eps_col = sbc.tile([P, 1], F32, tag="eps_col")
        nc.vector.memset(eps_col[:], EPS)

        # ---- internal DRAM (fresh per rep: Shared tiles allow one writer)
        def alloc_dram(rep):
            d = {}
            sfx = f"_r{rep}"
            d["zdr"] = dram.tile([T, P, D], F32, name="zdr" + sfx)
            d["ag2_in"] = dram.tile([T * P, BIGROW], F32, name="ag2i" + sfx)
            d["table2"] = dram.tile([NV, BIGROW], F32, name="tb2" + sfx,
                                    addr_space="Shared")
            d["ags2_in"] = dram.tile([T * P, SMROW], F32, name="ags2i" + sfx)
            d["small2"] = dram.tile([NV, SMROW], F32, name="sm2" + sfx,
                                    addr_space="Shared")
            d["ag3_in"] = dram.tile([T * P, BIGROW3], F32, name="ag3i" + sfx)
            d["table3"] = dram.tile([NV, BIGROW3], F32, name="tb3" + sfx,
                                    addr_space="Shared")
            d["ags3_in"] = dram.tile([T * P, SMROW], F32, name="ags3i" + sfx)
            d["small3"] = dram.tile([NV, SMROW], F32, name="sm3" + sfx,
                                    addr_space="Shared")
            d["st_io"] = [
                (dram.tile([P, 8], F32, name=f"st_in{i}" + sfx),
                 dram.tile([P, 8], F32, name=f"st_out{i}" + sfx,
                           addr_space="Shared"))
                for i in range(2)]
            d["pool_in"] = dram.tile([G, HID], F32, name="pool_in" + sfx)
            d["pool_out"] = dram.tile([G, HID], F32, name="pool_out" + sfx,
                                      addr_space="Shared")
            return d

        dcur = {}

        def dget(name):
            return dcur[name]

        # ------------------------------------------------------------------
        def edge_ex(work, t, as_ap, ad_ap, nh):
            nch = cfg.nch[t]
            u = work.tile([P, nch, nh], F32, tag="u")
            nc.vector.tensor_tensor(out=u[:], in0=as_ap, in1=ad_ap,
                                    op=mybir.AluOpType.add)
            u2 = work.tile([P, nch, nh], F32, tag="u2")
            nc.vector.tensor_scalar(out=u2[:], in0=u[:], scalar1=0.2,
                                    scalar2=None, op0=mybir.AluOpType.mult)
            lr = work.tile([P, nch, nh], F32, tag="lr")
            nc.vector.tensor_tensor(out=lr[:], in0=u[:], in1=u2[:],
                                    op=mybir.AluOpType.max)
            ex = work.tile([P, nch, nh], F32, tag="ex")
            nc.scalar.activation(ex[:], lr[:], mybir.ActivationFunctionType.Exp)
            return ex

        def build_T(work, t):
            nch = cfg.nch[t]
            c0 = cfg.c_off[t]
            T_ = work.tile([P, nch, P], F32, tag="Tsel")
            nc.vector.tensor_tensor(
                out=T_[:],
                in0=iota[:].unsqueeze(1).to_broadcast([P, nch, P]),
                in1=dstloc[:, c0:c0 + nch].unsqueeze(2).to_broadcast([P, nch, P]),
                op=mybir.AluOpType.is_equal)
            return T_

        def recip_den(work, den_ap, nh):
            den_c = work.tile([P, nh], F32, tag="den_c")
            nc.vector.tensor_scalar(out=den_c[:], in0=den_ap, scalar1=1e-30,
                                    scalar2=None, op0=mybir.AluOpType.max)
            recip = work.tile([P, nh], F32, tag="recip")
            nc.vector.reciprocal(recip[:], den_c[:])
            return recip

        def tail_stats_zT(work, psum, t, z_sb, stats_acc):
            z2 = work.tile([P, D], F32, tag="z2")
            nc.scalar.activation(z2[:], z_sb[:],
                                 mybir.ActivationFunctionType.Square)
            stq = psum.tile([P, 8], F32, space="PSUM", tag="stq")
            for b in range(4):
                nc.tensor.matmul(out=stq[:, b:b + 1],
                                 lhsT=z_sb[:, b * P:(b + 1) * P],
                                 rhs=ones_col[:], start=True, stop=True)
                nc.tensor.matmul(out=stq[:, 4 + b:5 + b],
                                 lhsT=z2[:, b * P:(b + 1) * P],
                                 rhs=ones_col[:], start=True, stop=True)
            nc.vector.tensor_tensor(out=stats_acc[:], in0=stats_acc[:],
                                    in1=stq[:], op=mybir.AluOpType.add)
            zT = work.tile([P, D], F32, tag="zT")
            for b in range(4):
                ztp = psum.tile([P, P], F32, space="PSUM", tag="ztp")
                nc.tensor.transpose(out=ztp[:], in_=z_sb[:, b * P:(b + 1) * P],
                                    identity=ident[:])
                nc.vector.tensor_copy(out=zT[:, b * P:(b + 1) * P], in_=ztp[:])
            nc.sync.dma_start(dget("zdr")[t], zT[:])

        def phase_a_tile(work, psum, layer, t, stats_ps, pool_ps):
            nch = cfg.nch[t]
            NE = nch * P
            s0 = cfg.s_off[t]
            ssl = slice(s0, s0 + NE // 16)
            nh = H if layer < 3 else 1

            if layer == 1:
                Gs = work.tile([P, nch, SMROW], F32, tag="Gs")
                nc.gpsimd.dma_gather(Gs[:], small1_d[:], idx_src[:, ssl],
                                     NE, NE, SMROW, single_packet=False)
                Gd = work.tile([P, nch, SMROW], F32, tag="Gd")
                nc.gpsimd.dma_gather(Gd[:], small1_d[:], idx_dst[:, ssl],
                                     NE, NE, SMROW, single_packet=False)
                as_ap = Gs[:, :, 0:4]
                ad_ap = Gd[:, :, 4:8]
            else:
                tab = dget("table2") if layer == 2 else dget("table3")
                sml = dget("small2") if layer == 2 else dget("small3")
                brow = BIGROW if layer == 2 else BIGROW3
                dd = D if layer == 2 else HID
                Gb = work.tile([P, nch, brow], F32, tag="Gbig")
                nc.gpsimd.dma_gather(Gb[:], tab[:], idx_src[:, ssl],
                                     NE, NE, brow, single_packet=False)
                Gd = work.tile([P, nch, SMROW], F32, tag="Gd")
                nc.gpsimd.dma_gather(Gd[:], sml[:], idx_dst[:, ssl],
                                     NE, NE, SMROW, single_packet=False)
                as_ap = Gb[:, :, dd:dd + nh]
                ad_ap = Gd[:, :, 4:4 + nh]

            ex = edge_ex(work, t, as_ap, ad_ap, nh)
            T_ = build_T(work, t)

            if layer == 1:
                exd = work.tile([P, nch, H], F32, tag="exd")
                nc.vector.tensor_tensor(
                    out=exd[:], in0=ex[:],
                    in1=Gs[:, :, 8:9].to_broadcast([P, nch, H]),
                    op=mybir.AluOpType.mult)
                qd_ps = psum.tile([P, 8], F32, space="PSUM", tag="den")
                for c in range(nch):
                    nc.tensor.matmul(out=qd_ps[:, 0:4], lhsT=T_[:, c, :],
                                     rhs=ex[:, c, :], start=(c == 0),
                                     stop=(c == nch - 1))
                for c in range(nch):
                    nc.tensor.matmul(out=qd_ps[:, 4:8], lhsT=T_[:, c, :],
                                     rhs=exd[:, c, :], start=(c == 0),
                                     stop=(c == nch - 1))
                recip = recip_den(work, qd_ps[:, 0:4], H)
                t4 = work.tile([P, H], F32, tag="t4")
                nc.vector.tensor_tensor(out=t4[:], in0=qd_ps[:, 4:8],
                                        in1=recip[:], op=mybir.AluOpType.mult)
                tT_ps = psum.tile([H, P], F32, space="PSUM", tag="ztp")
                nc.tensor.transpose(out=tT_ps[:], in_=t4[:], identity=ident[:])
                tT = work.tile([H, P], F32, tag="tTs")
                nc.vector.tensor_copy(out=tT[:], in_=tT_ps[:])
                z_ps = psum.tile([P, D], F32, space="PSUM", tag="msg")
                nc.tensor.matmul(out=z_ps[:], lhsT=tT[:], rhs=w1rows[:],
                                 start=True, stop=True)
                z_sb = work.tile([P, D], F32, tag="z")
                nc.vector.tensor_copy(out=z_sb[:], in_=z_ps[:])
                tail_stats_zT(work, psum, t, z_sb, stats_ps)
            elif layer == 2:
                msg_ps = psum.tile([P, D], F32, space="PSUM", tag="msg")
                den_ps = psum.tile([P, H], F32, space="PSUM", tag="den")
                for h in range(H):
                    Sh = work.tile([P, nch, P], F32, tag="Sh")
                    nc.vector.tensor_tensor(
                        out=Sh[:], in0=T_[:],
                        in1=ex[:, :, h:h + 1].to_broadcast([P, nch, P]),
                        op=mybir.AluOpType.mult)
                    for c in range(nch):
                        nc.tensor.matmul(
                            out=msg_ps[:, h * P:(h + 1) * P], lhsT=Sh[:, c, :],
                            rhs=Gb[:, c, h * P:(h + 1) * P],
                            start=(c == 0), stop=(c == nch - 1))
                for c in range(nch):
                    nc.tensor.matmul(out=den_ps[:], lhsT=T_[:, c, :],
                                     rhs=ex[:, c, :], start=(c == 0),
                                     stop=(c == nch - 1))
                recip = recip_den(work, den_ps[:], H)
                z_sb = work.tile([P, D], F32, tag="z")
                for h in range(H):
                    nc.vector.tensor_scalar(
                        out=z_sb[:, h * P:(h + 1) * P],
                        in0=msg_ps[:, h * P:(h + 1) * P],
                        scalar1=recip[:, h:h + 1], scalar2=None,
                        op0=mybir.AluOpType.mult)
                tail_stats_zT(work, psum, t, z_sb, stats_ps)
            else:
                msg_ps = psum.tile([P, HID], F32, space="PSUM", tag="msg")
                den_ps = psum.tile([P, H], F32, space="PSUM", tag="den")
                Sh = work.tile([P, nch, P], F32, tag="Sh")
                nc.vector.tensor_tensor(
                    out=Sh[:], in0=T_[:],
                    in1=ex[:, :, 0:1].to_broadcast([P, nch, P]),
                    op=mybir.AluOpType.mult)
                for c in range(nch):
                    nc.tensor.matmul(out=msg_ps[:], lhsT=Sh[:, c, :],
                                     rhs=Gb[:, c, 0:HID],
                                     start=(c == 0), stop=(c == nch - 1))
                for c in range(nch):
                    nc.tensor.matmul(out=den_ps[:, 0:1], lhsT=T_[:, c, :],
                                     rhs=ex[:, c, :], start=(c == 0),
                                     stop=(c == nch - 1))
                recip = recip_den(work, den_ps[:, 0:1], 1)
                z_sb = work.tile([P, HID], F32, tag="z3")
                nc.vector.tensor_scalar(out=z_sb[:], in0=msg_ps[:],
                                        scalar1=recip[:, 0:1], scalar2=None,
                                        op0=mybir.AluOpType.mult)
                h3 = work.tile([P, HID], F32, tag="h3")
                nc.vector.tensor_tensor(out=h3[:], in0=z_sb[:], in1=b3bc[:],
                                        op=mybir.AluOpType.add)
                h3r = work.tile([P, HID], F32, tag="h3r")
                nc.scalar.activation(h3r[:], h3[:],
                                     mybir.ActivationFunctionType.Relu)
                pq = psum.tile([G, HID], F32, space="PSUM", tag="pq")
                nc.tensor.matmul(out=pq[:], lhsT=ptile[:, t, :],
                                 rhs=h3r[:], start=True, stop=True)
                nc.vector.tensor_tensor(out=pool_ps[:], in0=pool_ps[:],
                                        in1=pq[:], op=mybir.AluOpType.add)

        def finish_stats(stats_acc, gbe, sio):
            sin, sout = sio
            nc.sync.dma_start(sin[:], stats_acc[:])
            nc.gpsimd.collective_compute(
                "AllReduce", mybir.AluOpType.add, replica_groups=rg,
                ins=[sin[:].opt()], outs=[sout[:].opt()])
            stg = wsm.tile([P, 8], F32, tag="stg")
            nc.sync.dma_start(stg[:], sout[:])
            inv = 1.0 / cfg.N
            mean = wsm.tile([P, 4], F32, tag="bn_mean")
            nc.vector.tensor_scalar(out=mean[:], in0=stg[:, 0:4], scalar1=inv,
                                    scalar2=None, op0=mybir.AluOpType.mult)
            var = wsm.tile([P, 4], F32, tag="bn_var")
            nc.vector.tensor_scalar(out=var[:], in0=stg[:, 4:8], scalar1=inv,
                                    scalar2=None, op0=mybir.AluOpType.mult)
            mu2 = wsm.tile([P, 4], F32, tag="bn_mu2")
            nc.vector.tensor_tensor(out=mu2[:], in0=mean[:], in1=mean[:],
                                    op=mybir.AluOpType.mult)
            nc.vector.tensor_tensor(out=var[:], in0=var[:], in1=mu2[:],
                                    op=mybir.AluOpType.subtract)
            sd = wsm.tile([P, 4], F32, tag="bn_sd")
            nc.scalar.activation(sd[:], var[:],
                                 mybir.ActivationFunctionType.Sqrt,
                                 bias=eps_col[:])
            rcp = wsm.tile([P, 4], F32, tag="bn_rcp")
            nc.vector.reciprocal(rcp[:], sd[:])
            scale = wsm.tile([P, 4], F32, tag="bn_scale")
            nc.vector.tensor_tensor(out=scale[:], in0=gbe[:, 0:4], in1=rcp[:],
                                    op=mybir.AluOpType.mult)
            msc = wsm.tile([P, 4], F32, tag="bn_msc")
            nc.vector.tensor_tensor(out=msc[:], in0=mean[:], in1=scale[:],
                                    op=mybir.AluOpType.mult)
            shift = wsm.tile([P, 4], F32, tag="bn_shift")
            nc.vector.tensor_tensor(out=shift[:], in0=gbe[:, 4:8], in1=msc[:],
                                    op=mybir.AluOpType.subtract)
            return scale, shift

        def phase_b_tile(work, psum, t, wpre, bcol, d_next, n_as, brow,
                         agb, ags, scale, shift):
            zT = work.tile([P, D], F32, tag="zTb")
            nc.sync.dma_start(zT[:], dget("zdr")[t])
            hbT = work.tile([P, D], F32, tag="hbT")
            for b in range(4):
                nc.scalar.activation(hbT[:, b * P:(b + 1) * P],
                                     zT[:, b * P:(b + 1) * P],
                                     mybir.ActivationFunctionType.Relu,
                                     bias=shift[:, b:b + 1],
                                     scale=scale[:, b:b + 1])
            hp_ps = psum.tile([P, d_next], F32, space="PSUM", tag="hp")
            ab_ps = psum.tile([P, 8], F32, space="PSUM", tag="ab")
            for b in range(4):
                nc.tensor.matmul(out=hp_ps[:], lhsT=hbT[:, b * P:(b + 1) * P],
                                 rhs=wpre[:, b, :], start=(b == 0),
                                 stop=(b == 3))
            for b in range(4):
                nc.tensor.matmul(out=ab_ps[:, 0:2 * n_as],
                                 lhsT=hbT[:, b * P:(b + 1) * P],
                                 rhs=bcol[:, b, :], start=(b == 0),
                                 stop=(b == 3))
            stage = work.tile([P, brow], F32, tag="stage")
            nc.vector.memset(stage[:, d_next + n_as:brow], 0.0)
            nc.vector.tensor_scalar(out=stage[:, 0:d_next], in0=hp_ps[:],
                                    scalar1=maskt[:, t:t + 1], scalar2=None,
                                    op0=mybir.AluOpType.mult)
            nc.vector.tensor_scalar(out=stage[:, d_next:d_next + n_as],
                                    in0=ab_ps[:, 0:n_as],
                                    scalar1=maskt[:, t:t + 1], scalar2=None,
                                    op0=mybir.AluOpType.mult)
            sms = work.tile([P, SMROW], F32, tag="sms")
            nc.vector.memset(sms[:], 0.0)
            nc.vector.tensor_scalar(out=sms[:, 4:4 + n_as],
                                    in0=ab_ps[:, n_as:2 * n_as],
                                    scalar1=maskt[:, t:t + 1], scalar2=None,
                                    op0=mybir.AluOpType.mult)
            nc.sync.dma_start(agb[t * P:(t + 1) * P, :], stage[:])
            nc.sync.dma_start(ags[t * P:(t + 1) * P, :], sms[:])

        def run_phase_a(layer):
            with ExitStack() as ps:
                work = ps.enter_context(
                    tc.tile_pool(name=f"wA{layer}", bufs=2))
                psum = ps.enter_context(
                    tc.tile_pool(name=f"pA{layer}", bufs=2, space="PSUM"))
                if layer < 3:
                    stats_acc = work.tile([P, 8], F32, tag="stats_acc")
                    nc.vector.memset(stats_acc[:], 0.0)
                    for t in range(T):
                        phase_a_tile(work, psum, layer, t, stats_acc, None)
                    gbe = gbe1 if layer == 1 else gbe2
                    return finish_stats(stats_acc, gbe, dget("st_io")[layer - 1])
                else:
                    pool_acc = wsm.tile([G, HID], F32, tag="pool_acc")
                    nc.vector.memset(pool_acc[:], 0.0)
                    for t in range(T):
                        phase_a_tile(work, psum, layer, t, None, pool_acc)
                    pool_sb = wsm.tile([G, HID], F32, tag="pool_sb")
                    nc.vector.tensor_scalar(out=pool_sb[:], in0=pool_acc[:],
                                            scalar1=rcnt[:], scalar2=None,
                                            op0=mybir.AluOpType.mult)
                    nc.sync.dma_start(dget("pool_in")[:], pool_sb[:])
                    return None

        def run_phase_b(layer, scale, shift):
            wpre = w2pre if layer == 1 else w3pre
            bcol = b2c if layer == 1 else b3c
            d_next = D if layer == 1 else HID
            n_as = 4 if layer == 1 else 1
            brow = BIGROW if layer == 1 else BIGROW3
            agb = dget("ag2_in") if layer == 1 else dget("ag3_in")
            ags = dget("ags2_in") if layer == 1 else dget("ags3_in")
            tab = dget("table2") if layer == 1 else dget("table3")
            sml = dget("small2") if layer == 1 else dget("small3")
            with ExitStack() as ps:
                work = ps.enter_context(
                    tc.tile_pool(name=f"wB{layer}", bufs=2))
                psum = ps.enter_context(
                    tc.tile_pool(name=f"pB{layer}", bufs=2, space="PSUM"))
                for t in range(T):
                    phase_b_tile(work, psum, t, wpre, bcol, d_next, n_as,
                                 brow, agb, ags, scale, shift)
            nc.gpsimd.collective_compute(
                "AllGather", mybir.AluOpType.bypass, replica_groups=rg,
                ins=[agb[:].opt()], outs=[tab[:].opt()])
            nc.gpsimd.collective_compute(
                "AllGather", mybir.AluOpType.bypass, replica_groups=rg,
                ins=[ags[:].opt()], outs=[sml[:].opt()])

        # ================== program ==================
        for _rep in range(reps):
            dcur.clear()
            dcur.update(alloc_dram(_rep))
            scale1, shift1 = run_phase_a(1)
            run_phase_b(1, scale1, shift1)
            scale2, shift2 = run_phase_a(2)
            run_phase_b(2, scale2, shift2)
            run_phase_a(3)

            nc.gpsimd.collective_compute(
                "AllReduce", mybir.AluOpType.add, replica_groups=rg,
                ins=[dget("pool_in")[:].opt()],
                outs=[dget("pool_out")[:].opt()])

        with ExitStack() as ps:
            psum = ps.enter_context(
                tc.tile_pool(name="pMLP", bufs=1, space="PSUM"))
            poolg = wsm.tile([G, HID], F32, tag="poolg")
            nc.sync.dma_start(poolg[:], dget("pool_out")[:])
            pT_ps = psum.tile([HID, G], F32, space="PSUM", tag="pT")
            nc.tensor.transpose(out=pT_ps[:], in_=poolg[:],
                                identity=ident[:G, :G])
            pT = wsm.tile([HID, G], F32, tag="pTs")
            nc.vector.tensor_copy(out=pT[:], in_=pT_ps[:])
            m1_ps = psum.tile([G, HID // 2], F32, space="PSUM", tag="m1")
            nc.tensor.matmul(out=m1_ps[:], lhsT=pT[:], rhs=lw1T[:],
                             start=True, stop=False)
            nc.tensor.matmul(out=m1_ps[:], lhsT=ones_row[:, 0:G], rhs=lb1r[:],
                             start=False, stop=True)
            m1 = wsm.tile([G, HID // 2], F32, tag="m1s")
            nc.scalar.activation(m1[:], m1_ps[:],
                                 mybir.ActivationFunctionType.Relu)
            m1T_ps = psum.tile([HID // 2, G], F32, space="PSUM", tag="m1T")
            nc.tensor.transpose(out=m1T_ps[:], in_=m1[:],
                                identity=ident[:G, :G])
            m1T = wsm.tile([HID // 2, G], F32, tag="m1Ts")
            nc.vector.tensor_copy(out=m1T[:], in_=m1T_ps[:])
            o_ps = psum.tile([G, CLS], F32, space="PSUM", tag="o")
            nc.tensor.matmul(out=o_ps[:], lhsT=m1T[:], rhs=lw2T[:],
                             start=True, stop=False)
            nc.tensor.matmul(out=o_ps[:], lhsT=ones_row[:, 0:G], rhs=lb2r[:],
                             start=False, stop=True)
            o_sb = wsm.tile([G, CLS], F32, tag="o_sb")
            nc.vector.tensor_copy(out=o_sb[:], in_=o_ps[:])
            nc.sync.dma_start(out_d[:], o_sb[:])

    return nc


# ----------------------------------------------------------------------------
# host-side preparation
# ----------------------------------------------------------------------------

def wrap_idx(vals, S):
    n = len(vals)
    a = np.zeros((16, S), np.int16)
    ii = np.arange(n)
    a[ii % 16, ii // 16] = vals.astype(np.int16)
    return np.tile(a, (8, 1))


def make_cfg_and_inputs(inputs):
    x = np.asarray(inputs["x"], np.float32).reshape(-1)
    ei = np.asarray(inputs["edge_index"]).astype(np.int64)
    batch = np.asarray(inputs["batch"]).astype(np.int64)
    N = x.shape[0]
    T = int(np.ceil(N / (NCORES * P)))
    NV = NCORES * T * P
    pernode = T * P

    loop = np.arange(N, dtype=np.int64)
    src = np.concatenate([ei[0], loop])
    dst = np.concatenate([ei[1], loop])
    E = src.shape[0]

    order = np.argsort(dst, kind="stable")
    src_s = src[order]
    dst_s = dst[order]
    owner = dst_s // pernode
    tile_id = (dst_s % pernode) // P

    counts = np.zeros((NCORES, T), np.int64)
    np.add.at(counts, (owner, tile_id), 1)
    ne_t = np.maximum(counts.max(axis=0), 1)
    ne_t = ((ne_t + P - 1) // P * P).astype(np.int64)
    nch = (ne_t // P).astype(np.int64)

    cfg = Cfg(N, E, nch.tolist())
    PAD = NV - 1

    idx_src_all, idx_dst_all, dstloc_all = [], [], []
    for c in range(NCORES):
        isrc = np.zeros((P, cfg.sumS), np.int16)
        idst = np.zeros((P, cfg.sumS), np.int16)
        dloc = np.full((P, cfg.sumC), 999.0, np.float32)
        base = c * pernode
        for t in range(T):
            lo = np.searchsorted(dst_s, base + t * P)
            hi = np.searchsorted(dst_s, base + (t + 1) * P)
            n = hi - lo
            NE = int(ne_t[t])
            sv = np.full(NE, PAD, np.int64)
            dv = np.full(NE, PAD, np.int64)
            dl = np.full(NE, 999.0, np.float32)
            sv[:n] = src_s[lo:hi]
            dv[:n] = dst_s[lo:hi]
            dl[:n] = (dst_s[lo:hi] - base - t * P).astype(np.float32)
            s0 = cfg.s_off[t]
            isrc[:, s0:s0 + NE // 16] = wrap_idx(sv, NE // 16)
            idst[:, s0:s0 + NE // 16] = wrap_idx(dv, NE // 16)
            c0 = cfg.c_off[t]
            dloc[:, c0:c0 + int(nch[t])] = dl.reshape(int(nch[t]), P).T
        idx_src_all.append(isrc)
        idx_dst_all.append(idst)
        dstloc_all.append(dloc)

    W1 = np.asarray(inputs["W1"], np.float32)
    as1 = np.asarray(inputs["as1"], np.float32)
    ad1 = np.asarray(inputs["ad1"], np.float32)
    W2 = np.asarray(inputs["W2"], np.float32)
    as2 = np.asarray(inputs["as2"], np.float32)
    ad2 = np.asarray(inputs["ad2"], np.float32)
    W3 = np.asarray(inputs["W3"], np.float32)
    as3 = np.asarray(inputs["as3"], np.float32)
    ad3 = np.asarray(inputs["ad3"], np.float32)
    g1 = np.asarray(inputs["g1"], np.float32)
    be1 = np.asarray(inputs["be1"], np.float32)
    g2 = np.asarray(inputs["g2"], np.float32)
    be2 = np.asarray(inputs["be2"], np.float32)
    b3 = np.asarray(inputs["b3"], np.float32)
    lw1 = np.asarray(inputs["lw1"], np.float32)
    lb1 = np.asarray(inputs["lb1"], np.float32)
    lw2 = np.asarray(inputs["lw2"], np.float32)
    lb2 = np.asarray(inputs["lb2"], np.float32)

    w1col = W1[:, 0]
    s1 = (w1col.reshape(H, HID) * as1).sum(1)
    d1 = (w1col.reshape(H, HID) * ad1).sum(1)

    small1 = np.zeros((NV, SMROW), np.float32)
    small1[:N, 0:4] = x[:, None] * s1[None, :]
    small1[:N, 4:8] = x[:, None] * d1[None, :]
    small1[:N, 8] = x

    w1rows = np.zeros((H, D), np.float32)
    for h in range(H):
        w1rows[h, h * HID:(h + 1) * HID] = w1col[h * HID:(h + 1) * HID]

    W2T = np.ascontiguousarray(W2.T)
    As2 = np.zeros((D, H), np.float32)
    Ad2 = np.zeros((D, H), np.float32)
    for h in range(H):
        As2[h * HID:(h + 1) * HID, h] = as2[h]
        Ad2[h * HID:(h + 1) * HID, h] = ad2[h]
    Bs2 = W2T @ As2
    Bd2 = W2T @ Ad2
    w2pre = np.ascontiguousarray(W2T.reshape(4, P, D).transpose(1, 0, 2))
    b2c = np.ascontiguousarray(
        np.concatenate([Bs2, Bd2], 1).reshape(4, P, 8).transpose(1, 0, 2))

    W3T = np.ascontiguousarray(W3.T)
    Bs3 = W3T @ as3.T
    Bd3 = W3T @ ad3.T
    w3pre = np.ascontiguousarray(W3T.reshape(4, P, HID).transpose(1, 0, 2))
    b3c = np.ascontiguousarray(
        np.concatenate([Bs3, Bd3], 1).reshape(4, P, 2).transpose(1, 0, 2))

    gbe1 = np.concatenate([g1.reshape(4, P).T, be1.reshape(4, P).T], 1)
    gbe2 = np.concatenate([g2.reshape(4, P).T, be2.reshape(4, P).T], 1)
    b3bc = np.tile(b3[None, :], (P, 1)).astype(np.float32)

    cnt = np.bincount(batch, minlength=G).astype(np.float32)
    rcnt = (1.0 / np.maximum(cnt, 1.0)).reshape(G, 1).astype(np.float32)

    ptile_all, mask_all = [], []
    for c in range(NCORES):
        pt = np.zeros((P, T, G), np.float32)
        mk = np.zeros((P, T), np.float32)
        base = c * pernode
        for t in range(T):
            ids = base + t * P + np.arange(P)
            real = ids < N
            mk[real, t] = 1.0
            bb = batch[ids[real]]
            pt[np.arange(P)[real], t, bb] = 1.0
        ptile_all.append(pt)
        mask_all.append(mk)

    iota = np.tile(np.arange(P, dtype=np.float32)[None, :], (P, 1))

    common = dict(
        small1=small1,
        iota=np.ascontiguousarray(iota, np.float32),
        w1rows=w1rows,
        w2pre=w2pre.astype(np.float32), b2c=b2c.astype(np.float32),
        w3pre=w3pre.astype(np.float32), b3c=b3c.astype(np.float32),
        gbe1=gbe1.astype(np.float32), gbe2=gbe2.astype(np.float32),
        b3bc=b3bc,
        rcnt=rcnt,
        lw1T=np.ascontiguousarray(lw1.T, np.float32),
        lb1r=lb1.reshape(1, -1).astype(np.float32),
        lw2T=np.ascontiguousarray(lw2.T, np.float32),
        lb2r=lb2.reshape(1, -1).astype(np.float32),
    )
    in_maps = []
    for c in range(NCORES):
        m = dict(common)
        m["idx_src"] = idx_src_all[c]
        m["idx_dst"] = idx_dst_all[c]
        m["dstloc"] = dstloc_all[c]
        m["ptile"] = ptile_all[c]
        m["maskt"] = mask_all[c]
        in_maps.append(m)
    return cfg, in_maps


# ----------------------------------------------------------------------------
# entry point
# ----------------------------------------------------------------------------

_CACHE = {}


def _get_program(cfg):
    key = cfg.key()
    if key not in _CACHE:
        nc = build_program(cfg)
        nc.compile()
        _CACHE[key] = nc
    return _CACHE[key]


def kernel(**inputs):
    cfg, in_maps = make_cfg_and_inputs(inputs)
    nc = _get_program(cfg)
    res = run_bass_kernel_spmd(nc, in_maps, core_ids=list(range(NCORES)))
    return np.asarray(res.results[0]["out"])


# revision 27
# speedup vs baseline: 30.9162x; 1.4082x over previous
"""Trainium2 Bass kernel for BioGNN (3-layer GAT + BN + global mean pool + MLP).

Distribution (8 NeuronCores, SPMD):
  - Nodes padded to NV = 8*T*128, sharded contiguously; core c owns T
    dst-tiles of 128 nodes.
  - Edges (incl. self-loops) partitioned by owner(dst), sorted by dst;
    per (core, dst-tile) edge lists are padded to a per-tile-uniform
    multiple of 128 (max over cores) so all cores run one program. Pad
    edges point at an all-zero table row and dst_local=999 so they
    contribute exactly zero.
  - Per dst-tile: dma_gather pulls per-edge rows [hp | a_src] (by src)
    and [a_dst] (by dst) from replicated HBM tables; ex =
    exp(leakyrelu(a_s+a_d)); a 0/1 selector T (iota==dst_local, built on
    VectorE) scaled by ex gives S_h; TensorE matmuls accumulate segment
    sums (messages) and softmax denominators in PSUM. Max-subtraction is
    skipped (it cancels exactly; logits are O(1) so fp32 exp is safe).
  - BN: per-core stats via thin matmuls (pads are exactly zero), 4KB
    AllReduce, applied fused with ReLU on ScalarE in transposed layout;
    next layer's table hp = relu(bn(z)) @ W^T computed per shard,
    AllGathered (table + small a_s/a_d table).
  - Mean pool via per-tile batch-selector matmul + tiny AllReduce; MLP
    head replicated.
"""
import numpy as np
from contextlib import ExitStack

import concourse.bass as bass
import concourse.tile as tile
from concourse import bacc, mybir
from concourse.bass_utils import run_bass_kernel_spmd
from concourse.masks import make_identity

P = 128
F32 = mybir.dt.float32
BF16 = mybir.dt.bfloat16
I16 = mybir.dt.int16
BF16_TABLES = True
NCORES = 8
HID = 128
H = 4
D = H * HID            # 512
# big-table rows: hp in TDT (bf16 or f32) + a_src stored as raw f32 bytes + pad
if BF16_TABLES:
    TDT = BF16
    BIGROW = 640       # bf16 elems: 512 hp + 8 (4 f32 a_s) + pad = 1280B
    BIGROW3 = 256      # bf16 elems: 128 hp + 2 (1 f32 a_s) + pad = 512B
    ASW = 2            # a_s f32 occupies 2 table elems
else:
    TDT = F32
    BIGROW = D + 64    # 576 f32 = 2304B
    BIGROW3 = HID + 64 # 192 f32 = 768B
    ASW = 1
SMROW = 128            # 512B rows for small tables (>=512B: line-rate)
CLS = 5
G = 50
EPS = 1e-5


class Cfg:
    def __init__(self, N, E, nch):
        self.N = N
        self.E = E
        self.nch = list(nch)
        self.T = len(nch)
        self.NV = NCORES * self.T * P
        self.s_off = np.cumsum([0] + [c * 8 for c in nch]).tolist()
        self.c_off = np.cumsum([0] + list(nch)).tolist()
        self.sumS = int(self.s_off[-1])
        self.sumC = int(self.c_off[-1])

    def key(self):
        return (self.N, self.E, tuple(self.nch))


# ----------------------------------------------------------------------------
# device program
# ----------------------------------------------------------------------------

def build_program(cfg, reps=1, upto="full"):
    nc = bacc.Bacc("TRN2", target_bir_lowering=False, debug=False,
                   num_devices=NCORES)
    T, NV = cfg.T, cfg.NV
    rg = [list(range(NCORES))]

    def di(name, shape, dtype=F32):
        return nc.dram_tensor(name, shape, dtype, kind="ExternalInput")

    small1_d = di("small1", [NV, SMROW])
    idx_src_d = di("idx_src", [P, cfg.sumS], I16)
    ad1loc_d = di("ad1loc", [cfg.T * P, 4])
    dstloc_d = di("dstloc", [P, cfg.sumC])
    iota_d = di("iota", [P, P])
    w1rows_d = di("w1rows", [H, D])
    w2pre_d = di("w2pre", [P, 4, D])
    b2_d = di("b2c", [P, 4, 8])
    w3pre_d = di("w3pre", [P, 4, HID])
    b3c_d = di("b3c", [P, 4, 2])
    gbe1_d = di("gbe1", [P, 8])
    gbe2_d = di("gbe2", [P, 8])
    b3bc_d = di("b3bc", [P, HID])
    ptile_d = di("ptile", [P, T, G])
    rcnt_d = di("rcnt", [G, 1])
    mask_d = di("maskt", [P, T])
    lw1T_d = di("lw1T", [HID, HID // 2])
    lb1_d = di("lb1r", [1, HID // 2])
    lw2T_d = di("lw2T", [HID // 2, CLS])
    lb2_d = di("lb2r", [1, CLS])

    out_d = nc.dram_tensor("out", [G, CLS], F32, kind="ExternalOutput")

    with ExitStack() as stk:
        tc = stk.enter_context(tile.TileContext(nc))
        sbc = stk.enter_context(tc.tile_pool(name="const", bufs=1))
        wsm = stk.enter_context(tc.tile_pool(name="wsm", bufs=2))
        dram = stk.enter_context(tc.tile_pool(name="dram", bufs=1, space="DRAM"))

        # ---- persistent SBUF constants
        def load(name, dr, shape, dtype=F32):
            t = sbc.tile(shape, dtype, tag=name)
            nc.sync.dma_start(t[:], dr[:])
            return t

        idx_src = load("idx_src", idx_src_d, [P, cfg.sumS], I16)
        dstloc = load("dstloc", dstloc_d, [P, cfg.sumC])
        iota = load("iota", iota_d, [P, P])
        w1rows = load("w1rows", w1rows_d, [H, D])
        w2pre = load("w2pre", w2pre_d, [P, 4, D])
        b2c = load("b2c", b2_d, [P, 4, 8])
        w3pre = load("w3pre", w3pre_d, [P, 4, HID])
        b3c = load("b3c", b3c_d, [P, 4, 2])
        gbe1 = load("gbe1", gbe1_d, [P, 8])
        gbe2 = load("gbe2", gbe2_d, [P, 8])
        b3bc = load("b3bc", b3bc_d, [P, HID])
        ptile = load("ptile", ptile_d, [P, T, G])
        rcnt = load("rcnt", rcnt_d, [G, 1])
        maskt = load("maskt", mask_d, [P, T])
        lw1T = load("lw1T", lw1T_d, [HID, HID // 2])
        lb1r = load("lb1r", lb1_d, [1, HID // 2])
        lw2T = load("lw2T", lw2T_d, [HID // 2, CLS])
        lb2r = load("lb2r", lb2_d, [1, CLS])

        ident = sbc.tile([P, P], F32, tag="ident")
        make_identity(nc, ident[:])
        ones_col = sbc.tile([P, 1], F32, tag="ones_col")
        nc.vector.memset(ones_col[:], 1.0)
        ones_row = sbc.tile([1, 64], F32, tag="ones_row")
        nc.vector.memset(ones_row[:], 1.0)
        eps_col = sbc.tile([P, 1], F32, tag="eps_col")
        nc.vector.memset(eps_col[:], EPS)

        # ---- internal DRAM (fresh per rep: Shared tiles allow one writer)
        def alloc_dram(rep):
            d = {}
            sfx = f"_r{rep}"
            d["zdr"] = dram.tile([T, P, D], F32, name="zdr" + sfx)
            d["ag2_in"] = dram.tile([T * P, BIGROW], TDT, name="ag2i" + sfx)
            d["table2"] = dram.tile([NV, BIGROW], TDT, name="tb2" + sfx,
                                    addr_space="Shared")
            d["adbuf2"] = dram.tile([T * P, 4], F32, name="adb2" + sfx)
            d["ag3_in"] = dram.tile([T * P, BIGROW3], TDT, name="ag3i" + sfx)
            d["table3"] = dram.tile([NV, BIGROW3], TDT, name="tb3" + sfx,
                                    addr_space="Shared")
            d["adbuf3"] = dram.tile([T * P, 4], F32, name="adb3" + sfx)
            d["st_io"] = [
                (dram.tile([P, 8], F32, name=f"st_in{i}" + sfx),
                 dram.tile([P, 8], F32, name=f"st_out{i}" + sfx,
                           addr_space="Shared"))
                for i in range(2)]
            d["pool_in"] = dram.tile([G, HID], F32, name="pool_in" + sfx)
            d["pool_out"] = dram.tile([G, HID], F32, name="pool_out" + sfx,
                                      addr_space="Shared")
            return d

        dcur = {}

        def dget(name):
            return dcur[name]

        # ------------------------------------------------------------------
        def edge_ex(work, t, as_ap, ad_ap, nh):
            nch = cfg.nch[t]
            u = work.tile([P, nch, nh], F32, tag="u")
            nc.vector.tensor_tensor(out=u[:], in0=as_ap, in1=ad_ap,
                                    op=mybir.AluOpType.add)
            u2 = work.tile([P, nch, nh], F32, tag="u2")
            nc.vector.tensor_scalar(out=u2[:], in0=u[:], scalar1=0.2,
                                    scalar2=None, op0=mybir.AluOpType.mult)
            lr = work.tile([P, nch, nh], F32, tag="lr")
            nc.vector.tensor_tensor(out=lr[:], in0=u[:], in1=u2[:],
                                    op=mybir.AluOpType.max)
            ex = work.tile([P, nch, nh], F32, tag="ex")
            nc.scalar.activation(ex[:], lr[:], mybir.ActivationFunctionType.Exp)
            return ex

        def build_T(work, t):
            nch = cfg.nch[t]
            c0 = cfg.c_off[t]
            T_ = work.tile([P, nch, P], F32, tag="Tsel")
            nc.vector.tensor_tensor(
                out=T_[:],
                in0=iota[:].unsqueeze(1).to_broadcast([P, nch, P]),
                in1=dstloc[:, c0:c0 + nch].unsqueeze(2).to_broadcast([P, nch, P]),
                op=mybir.AluOpType.is_equal)
            return T_

        def recip_den(work, den_ap, nh):
            den_c = work.tile([P, nh], F32, tag="den_c")
            nc.vector.tensor_scalar(out=den_c[:], in0=den_ap, scalar1=1e-30,
                                    scalar2=None, op0=mybir.AluOpType.max)
            recip = work.tile([P, nh], F32, tag="recip")
            nc.vector.reciprocal(recip[:], den_c[:])
            return recip

        def tail_stats_zT(work, psum, t, z_sb, stats_acc):
            z2 = work.tile([P, D], F32, tag="z2")
            nc.scalar.activation(z2[:], z_sb[:],
                                 mybir.ActivationFunctionType.Square)
            stq = psum.tile([P, 8], F32, space="PSUM", tag="sps")
            for b in range(4):
                nc.tensor.matmul(out=stq[:, b:b + 1],
                                 lhsT=z_sb[:, b * P:(b + 1) * P],
                                 rhs=ones_col[:], start=True, stop=True)
                nc.tensor.matmul(out=stq[:, 4 + b:5 + b],
                                 lhsT=z2[:, b * P:(b + 1) * P],
                                 rhs=ones_col[:], start=True, stop=True)
            nc.vector.tensor_tensor(out=stats_acc[:], in0=stats_acc[:],
                                    in1=stq[:], op=mybir.AluOpType.add)
            zT = work.tile([P, D], F32, tag="zT")
            for b in range(4):
                ztp = psum.tile([P, P], F32, space="PSUM", tag="ztp")
                nc.tensor.transpose(out=ztp[:], in_=z_sb[:, b * P:(b + 1) * P],
                                    identity=ident[:])
                nc.vector.tensor_copy(out=zT[:, b * P:(b + 1) * P], in_=ztp[:])
            nc.sync.dma_start(dget("zdr")[t], zT[:])

        def phase_a_tile(work, psum, layer, t, stats_ps, pool_ps):
            nch = cfg.nch[t]
            NE = nch * P
            s0 = cfg.s_off[t]
            ssl = slice(s0, s0 + NE // 16)
            nh = H if layer < 3 else 1

            if layer == 1:
                Gs = work.tile([P, nch, SMROW], F32, tag="Gs")
                nc.gpsimd.dma_gather(Gs[:], small1_d[:], idx_src[:, ssl],
                                     NE, NE, SMROW, single_packet=False)
                as_ap = Gs[:, :, 0:4]
                adsrc = ad1loc_d
            else:
                tab = dget("table2") if layer == 2 else dget("table3")
                brow = BIGROW if layer == 2 else BIGROW3
                dd = D if layer == 2 else HID
                Gb = work.tile([P, nch, brow], TDT, tag="Gbig")
                nc.gpsimd.dma_gather(Gb[:], tab[:], idx_src[:, ssl],
                                     NE, NE, brow, single_packet=False)
                if BF16_TABLES:
                    as_ap = Gb[:, :, dd:dd + ASW * nh].bitcast(F32)
                else:
                    as_ap = Gb[:, :, dd:dd + nh]
                adsrc = dget("adbuf2") if layer == 2 else dget("adbuf3")

            T_ = build_T(work, t)

            # per-edge a_d via local a_d column + selector transpose:
            # ad_e[e, h] = sum_j T[e, j] * ad_tile[j, h]
            ad_tile = work.tile([P, nh], F32, tag="ad_tile")
            nc.sync.dma_start(ad_tile[:], adsrc[t * P:(t + 1) * P, 0:nh])
            ad_ps = psum.tile([P, nch, nh], F32, space="PSUM", tag="sps")
            for c in range(nch):
                ttp = psum.tile([P, P], F32, space="PSUM", tag="ztp")
                nc.tensor.transpose(out=ttp[:], in_=T_[:, c, :],
                                    identity=ident[:])
                Tt = work.tile([P, P], F32, tag="Tt")
                nc.vector.tensor_copy(out=Tt[:], in_=ttp[:])
                nc.tensor.matmul(out=ad_ps[:, c, :], lhsT=Tt[:],
                                 rhs=ad_tile[:], start=True, stop=True)
            ad_ap = ad_ps[:]

            ex = edge_ex(work, t, as_ap, ad_ap, nh)

            if layer == 1:
                exd = work.tile([P, nch, H], F32, tag="exd")
                nc.vector.tensor_tensor(
                    out=exd[:], in0=ex[:],
                    in1=Gs[:, :, 8:9].to_broadcast([P, nch, H]),
                    op=mybir.AluOpType.mult)
                qd_ps = psum.tile([P, 8], F32, space="PSUM", tag="den")
                for c in range(nch):
                    nc.tensor.matmul(out=qd_ps[:, 0:4], lhsT=T_[:, c, :],
                                     rhs=ex[:, c, :], start=(c == 0),
                                     stop=(c == nch - 1))
                for c in range(nch):
                    nc.tensor.matmul(out=qd_ps[:, 4:8], lhsT=T_[:, c, :],
                                     rhs=exd[:, c, :], start=(c == 0),
                                     stop=(c == nch - 1))
                recip = recip_den(work, qd_ps[:, 0:4], H)
                t4 = work.tile([P, H], F32, tag="t4")
                nc.vector.tensor_tensor(out=t4[:], in0=qd_ps[:, 4:8],
                                        in1=recip[:], op=mybir.AluOpType.mult)
                tT_ps = psum.tile([H, P], F32, space="PSUM", tag="ztp")
                nc.tensor.transpose(out=tT_ps[:], in_=t4[:], identity=ident[:])
                tT = work.tile([H, P], F32, tag="tTs")
                nc.vector.tensor_copy(out=tT[:], in_=tT_ps[:])
                z_ps = psum.tile([P, D], F32, space="PSUM", tag="msg")
                nc.tensor.matmul(out=z_ps[:], lhsT=tT[:], rhs=w1rows[:],
                                 start=True, stop=True)
                z_sb = work.tile([P, D], F32, tag="z")
                nc.vector.tensor_copy(out=z_sb[:], in_=z_ps[:])
                tail_stats_zT(work, psum, t, z_sb, stats_ps)
            elif layer == 2:
                msg_ps = psum.tile([P, D], F32, space="PSUM", tag="msg")
                den_ps = psum.tile([P, H], F32, space="PSUM", tag="den")
                for h in range(H):
                    Sh = work.tile([P, nch, P], TDT, tag="Sh")
                    nc.vector.tensor_tensor(
                        out=Sh[:], in0=T_[:],
                        in1=ex[:, :, h:h + 1].to_broadcast([P, nch, P]),
                        op=mybir.AluOpType.mult)
                    for c in range(nch):
                        nc.tensor.matmul(
                            out=msg_ps[:, h * P:(h + 1) * P], lhsT=Sh[:, c, :],
                            rhs=Gb[:, c, h * P:(h + 1) * P],
                            start=(c == 0), stop=(c == nch - 1))
                for c in range(nch):
                    nc.tensor.matmul(out=den_ps[:], lhsT=T_[:, c, :],
                                     rhs=ex[:, c, :], start=(c == 0),
                                     stop=(c == nch - 1))
                recip = recip_den(work, den_ps[:], H)
                z_sb = work.tile([P, D], F32, tag="z")
                for h in range(H):
                    nc.vector.tensor_scalar(
                        out=z_sb[:, h * P:(h + 1) * P],
                        in0=msg_ps[:, h * P:(h + 1) * P],
                        scalar1=recip[:, h:h + 1], scalar2=None,
                        op0=mybir.AluOpType.mult)
                tail_stats_zT(work, psum, t, z_sb, stats_ps)
            else:
                msg_ps = psum.tile([P, HID], F32, space="PSUM", tag="msg")
                den_ps = psum.tile([P, H], F32, space="PSUM", tag="den")
                Sh = work.tile([P, nch, P], TDT, tag="Sh")
                nc.vector.tensor_tensor(
                    out=Sh[:], in0=T_[:],
                    in1=ex[:, :, 0:1].to_broadcast([P, nch, P]),
                    op=mybir.AluOpType.mult)
                for c in range(nch):
                    nc.tensor.matmul(out=msg_ps[:], lhsT=Sh[:, c, :],
                                     rhs=Gb[:, c, 0:HID],
                                     start=(c == 0), stop=(c == nch - 1))
                for c in range(nch):
                    nc.tensor.matmul(out=den_ps[:, 0:1], lhsT=T_[:, c, :],
                                     rhs=ex[:, c, :], start=(c == 0),
                                     stop=(c == nch - 1))
                recip = recip_den(work, den_ps[:, 0:1], 1)
                z_sb = work.tile([P, HID], F32, tag="z3")
                nc.vector.tensor_scalar(out=z_sb[:], in0=msg_ps[:],
                                        scalar1=recip[:, 0:1], scalar2=None,
                                        op0=mybir.AluOpType.mult)
                h3 = work.tile([P, HID], F32, tag="h3")
                nc.vector.tensor_tensor(out=h3[:], in0=z_sb[:], in1=b3bc[:],
                                        op=mybir.AluOpType.add)
                h3r = work.tile([P, HID], F32, tag="h3r")
                nc.scalar.activation(h3r[:], h3[:],
                                     mybir.ActivationFunctionType.Relu)
                pq = psum.tile([G, HID], F32, space="PSUM", tag="sps")
                nc.tensor.matmul(out=pq[:], lhsT=ptile[:, t, :],
                                 rhs=h3r[:], start=True, stop=True)
                nc.vector.tensor_tensor(out=pool_ps[:], in0=pool_ps[:],
                                        in1=pq[:], op=mybir.AluOpType.add)

        def finish_stats(stats_acc, gbe, sio):
            sin, sout = sio
            nc.sync.dma_start(sin[:], stats_acc[:])
            nc.gpsimd.collective_compute(
                "AllReduce", mybir.AluOpType.add, replica_groups=rg,
                ins=[sin[:].opt()], outs=[sout[:].opt()])
            stg = wsm.tile([P, 8], F32, tag="stg")
            nc.sync.dma_start(stg[:], sout[:])
            inv = 1.0 / cfg.N
            mean = wsm.tile([P, 4], F32, tag="bn_mean")
            nc.vector.tensor_scalar(out=mean[:], in0=stg[:, 0:4], scalar1=inv,
                                    scalar2=None, op0=mybir.AluOpType.mult)
            var = wsm.tile([P, 4], F32, tag="bn_var")
            nc.vector.tensor_scalar(out=var[:], in0=stg[:, 4:8], scalar1=inv,
                                    scalar2=None, op0=mybir.AluOpType.mult)
            mu2 = wsm.tile([P, 4], F32, tag="bn_mu2")
            nc.vector.tensor_tensor(out=mu2[:], in0=mean[:], in1=mean[:],
                                    op=mybir.AluOpType.mult)
            nc.vector.tensor_tensor(out=var[:], in0=var[:], in1=mu2[:],
                                    op=mybir.AluOpType.subtract)
            sd = wsm.tile([P, 4], F32, tag="bn_sd")
            nc.scalar.activation(sd[:], var[:],
                                 mybir.ActivationFunctionType.Sqrt,
                                 bias=eps_col[:])
            rcp = wsm.tile([P, 4], F32, tag="bn_rcp")
            nc.vector.reciprocal(rcp[:], sd[:])
            scale = wsm.tile([P, 4], F32, tag="bn_scale")
            nc.vector.tensor_tensor(out=scale[:], in0=gbe[:, 0:4], in1=rcp[:],
                                    op=mybir.AluOpType.mult)
            msc = wsm.tile([P, 4], F32, tag="bn_msc")
            nc.vector.tensor_tensor(out=msc[:], in0=mean[:], in1=scale[:],
                                    op=mybir.AluOpType.mult)
            shift = wsm.tile([P, 4], F32, tag="bn_shift")
            nc.vector.tensor_tensor(out=shift[:], in0=gbe[:, 4:8], in1=msc[:],
                                    op=mybir.AluOpType.subtract)
            return scale, shift

        def phase_b_tile(work, psum, t, wpre, bcol, d_next, n_as, brow,
                         agb, adb, scale, shift):
            zT = work.tile([P, D], F32, tag="zTb")
            nc.sync.dma_start(zT[:], dget("zdr")[t])
            hbT = work.tile([P, D], F32, tag="hbT")
            for b in range(4):
                nc.scalar.activation(hbT[:, b * P:(b + 1) * P],
                                     zT[:, b * P:(b + 1) * P],
                                     mybir.ActivationFunctionType.Relu,
                                     bias=shift[:, b:b + 1],
                                     scale=scale[:, b:b + 1])
            hp_ps = psum.tile([P, d_next], F32, space="PSUM", tag="hp")
            ab_ps = psum.tile([P, 8], F32, space="PSUM", tag="ab")
            for b in range(4):
                nc.tensor.matmul(out=hp_ps[:], lhsT=hbT[:, b * P:(b + 1) * P],
                                 rhs=wpre[:, b, :], start=(b == 0),
                                 stop=(b == 3))
            for b in range(4):
                nc.tensor.matmul(out=ab_ps[:, 0:2 * n_as],
                                 lhsT=hbT[:, b * P:(b + 1) * P],
                                 rhs=bcol[:, b, :], start=(b == 0),
                                 stop=(b == 3))
            stage = work.tile([P, brow], TDT, tag="stage")
            nc.vector.memset(stage[:, d_next + ASW * n_as:brow], 0.0)
            nc.vector.tensor_scalar(out=stage[:, 0:d_next], in0=hp_ps[:],
                                    scalar1=maskt[:, t:t + 1], scalar2=None,
                                    op0=mybir.AluOpType.mult)
            if BF16_TABLES:
                as_out = stage[:, d_next:d_next + ASW * n_as].bitcast(F32)
            else:
                as_out = stage[:, d_next:d_next + n_as]
            nc.vector.tensor_scalar(out=as_out,
                                    in0=ab_ps[:, 0:n_as],
                                    scalar1=maskt[:, t:t + 1], scalar2=None,
                                    op0=mybir.AluOpType.mult)
            ads = work.tile([P, n_as], F32, tag="ads")
            nc.vector.tensor_scalar(out=ads[:],
                                    in0=ab_ps[:, n_as:2 * n_as],
                                    scalar1=maskt[:, t:t + 1], scalar2=None,
                                    op0=mybir.AluOpType.mult)
            nc.sync.dma_start(agb[t * P:(t + 1) * P, :], stage[:])
            nc.sync.dma_start(adb[t * P:(t + 1) * P, 0:n_as], ads[:])

        def run_phase_a(layer):
            with ExitStack() as ps:
                work = ps.enter_context(
                    tc.tile_pool(name=f"wA{layer}", bufs=2))
                psum = ps.enter_context(
                    tc.tile_pool(name=f"pA{layer}", bufs=2, space="PSUM"))
                if layer < 3:
                    stats_acc = work.tile([P, 8], F32, tag="stats_acc")
                    nc.vector.memset(stats_acc[:], 0.0)
                    for t in range(T):
                        phase_a_tile(work, psum, layer, t, stats_acc, None)
                    gbe = gbe1 if layer == 1 else gbe2
                    return finish_stats(stats_acc, gbe, dget("st_io")[layer - 1])
                else:
                    pool_acc = wsm.tile([G, HID], F32, tag="pool_acc")
                    nc.vector.memset(pool_acc[:], 0.0)
                    for t in range(T):
                        phase_a_tile(work, psum, layer, t, None, pool_acc)
                    pool_sb = wsm.tile([G, HID], F32, tag="pool_sb")
                    nc.vector.tensor_scalar(out=pool_sb[:], in0=pool_acc[:],
                                            scalar1=rcnt[:], scalar2=None,
                                            op0=mybir.AluOpType.mult)
                    nc.sync.dma_start(dget("pool_in")[:], pool_sb[:])
                    return None

        def run_phase_b(layer, scale, shift):
            wpre = w2pre if layer == 1 else w3pre
            bcol = b2c if layer == 1 else b3c
            d_next = D if layer == 1 else HID
            n_as = 4 if layer == 1 else 1
            brow = BIGROW if layer == 1 else BIGROW3
            agb = dget("ag2_in") if layer == 1 else dget("ag3_in")
            adb = dget("adbuf2") if layer == 1 else dget("adbuf3")
            tab = dget("table2") if layer == 1 else dget("table3")
            with ExitStack() as ps:
                work = ps.enter_context(
                    tc.tile_pool(name=f"wB{layer}", bufs=2))
                psum = ps.enter_context(
                    tc.tile_pool(name=f"pB{layer}", bufs=2, space="PSUM"))
                for t in range(T):
                    phase_b_tile(work, psum, t, wpre, bcol, d_next, n_as,
                                 brow, agb, adb, scale, shift)
            nc.gpsimd.collective_compute(
                "AllGather", mybir.AluOpType.bypass, replica_groups=rg,
                ins=[agb[:].opt()], outs=[tab[:].opt()])

        # ================== program ==================
        stages = ["l1a", "l1b", "l2a", "l2b", "l3a", "full"]
        lim = stages.index(upto)
        for _rep in range(reps):
            dcur.clear()
            dcur.update(alloc_dram(_rep))
            scale1, shift1 = run_phase_a(1)
            if lim >= 1:
                run_phase_b(1, scale1, shift1)
            if lim >= 2:
                scale2, shift2 = run_phase_a(2)
            if lim >= 3:
                run_phase_b(2, scale2, shift2)
            if lim >= 4:
                run_phase_a(3)
            if lim >= 5:
                nc.gpsimd.collective_compute(
                    "AllReduce", mybir.AluOpType.add, replica_groups=rg,
                    ins=[dget("pool_in")[:].opt()],
                    outs=[dget("pool_out")[:].opt()])

        with ExitStack() as ps:
            psum = ps.enter_context(
                tc.tile_pool(name="pMLP", bufs=1, space="PSUM"))
            poolg = wsm.tile([G, HID], F32, tag="poolg")
            nc.sync.dma_start(poolg[:], dget("pool_out")[:])
            pT_ps = psum.tile([HID, G], F32, space="PSUM", tag="pT")
            nc.tensor.transpose(out=pT_ps[:], in_=poolg[:],
                                identity=ident[:G, :G])
            pT = wsm.tile([HID, G], F32, tag="pTs")
            nc.vector.tensor_copy(out=pT[:], in_=pT_ps[:])
            m1_ps = psum.tile([G, HID // 2], F32, space="PSUM", tag="m1")
            nc.tensor.matmul(out=m1_ps[:], lhsT=pT[:], rhs=lw1T[:],
                             start=True, stop=False)
            nc.tensor.matmul(out=m1_ps[:], lhsT=ones_row[:, 0:G], rhs=lb1r[:],
                             start=False, stop=True)
            m1 = wsm.tile([G, HID // 2], F32, tag="m1s")
            nc.scalar.activation(m1[:], m1_ps[:],
                                 mybir.ActivationFunctionType.Relu)
            m1T_ps = psum.tile([HID // 2, G], F32, space="PSUM", tag="m1T")
            nc.tensor.transpose(out=m1T_ps[:], in_=m1[:],
                                identity=ident[:G, :G])
            m1T = wsm.tile([HID // 2, G], F32, tag="m1Ts")
            nc.vector.tensor_copy(out=m1T[:], in_=m1T_ps[:])
            o_ps = psum.tile([G, CLS], F32, space="PSUM", tag="o")
            nc.tensor.matmul(out=o_ps[:], lhsT=m1T[:], rhs=lw2T[:],
                             start=True, stop=False)
            nc.tensor.matmul(out=o_ps[:], lhsT=ones_row[:, 0:G], rhs=lb2r[:],
                             start=False, stop=True)
            o_sb = wsm.tile([G, CLS], F32, tag="o_sb")
            nc.vector.tensor_copy(out=o_sb[:], in_=o_ps[:])
            nc.sync.dma_start(out_d[:], o_sb[:])

    return nc


# ----------------------------------------------------------------------------
# host-side preparation
# ----------------------------------------------------------------------------

def wrap_idx(vals, S):
    n = len(vals)
    a = np.zeros((16, S), np.int16)
    ii = np.arange(n)
    a[ii % 16, ii // 16] = vals.astype(np.int16)
    return np.tile(a, (8, 1))


def make_cfg_and_inputs(inputs):
    x = np.asarray(inputs["x"], np.float32).reshape(-1)
    ei = np.asarray(inputs["edge_index"]).astype(np.int64)
    batch = np.asarray(inputs["batch"]).astype(np.int64)
    N = x.shape[0]
    T = int(np.ceil(N / (NCORES * P)))
    NV = NCORES * T * P
    pernode = T * P

    loop = np.arange(N, dtype=np.int64)
    src = np.concatenate([ei[0], loop])
    dst = np.concatenate([ei[1], loop])
    E = src.shape[0]

    order = np.argsort(dst, kind="stable")
    src_s = src[order]
    dst_s = dst[order]
    owner = dst_s // pernode
    tile_id = (dst_s % pernode) // P

    counts = np.zeros((NCORES, T), np.int64)
    np.add.at(counts, (owner, tile_id), 1)
    ne_t = np.maximum(counts.max(axis=0), 1)
    ne_t = ((ne_t + P - 1) // P * P).astype(np.int64)
    nch = (ne_t // P).astype(np.int64)

    cfg = Cfg(N, E, nch.tolist())
    PAD = NV - 1

    idx_src_all, dstloc_all = [], []
    for c in range(NCORES):
        isrc = np.zeros((P, cfg.sumS), np.int16)
        dloc = np.full((P, cfg.sumC), 999.0, np.float32)
        base = c * pernode
        for t in range(T):
            lo = np.searchsorted(dst_s, base + t * P)
            hi = np.searchsorted(dst_s, base + (t + 1) * P)
            n = hi - lo
            NE = int(ne_t[t])
            sv = np.full(NE, PAD, np.int64)
            dl = np.full(NE, 999.0, np.float32)
            sv[:n] = src_s[lo:hi]
            dl[:n] = (dst_s[lo:hi] - base - t * P).astype(np.float32)
            s0 = cfg.s_off[t]
            isrc[:, s0:s0 + NE // 16] = wrap_idx(sv, NE // 16)
            c0 = cfg.c_off[t]
            dloc[:, c0:c0 + int(nch[t])] = dl.reshape(int(nch[t]), P).T
        idx_src_all.append(isrc)
        dstloc_all.append(dloc)

    W1 = np.asarray(inputs["W1"], np.float32)
    as1 = np.asarray(inputs["as1"], np.float32)
    ad1 = np.asarray(inputs["ad1"], np.float32)
    W2 = np.asarray(inputs["W2"], np.float32)
    as2 = np.asarray(inputs["as2"], np.float32)
    ad2 = np.asarray(inputs["ad2"], np.float32)
    W3 = np.asarray(inputs["W3"], np.float32)
    as3 = np.asarray(inputs["as3"], np.float32)
    ad3 = np.asarray(inputs["ad3"], np.float32)
    g1 = np.asarray(inputs["g1"], np.float32)
    be1 = np.asarray(inputs["be1"], np.float32)
    g2 = np.asarray(inputs["g2"], np.float32)
    be2 = np.asarray(inputs["be2"], np.float32)
    b3 = np.asarray(inputs["b3"], np.float32)
    lw1 = np.asarray(inputs["lw1"], np.float32)
    lb1 = np.asarray(inputs["lb1"], np.float32)
    lw2 = np.asarray(inputs["lw2"], np.float32)
    lb2 = np.asarray(inputs["lb2"], np.float32)

    w1col = W1[:, 0]
    s1 = (w1col.reshape(H, HID) * as1).sum(1)
    d1 = (w1col.reshape(H, HID) * ad1).sum(1)

    small1 = np.zeros((NV, SMROW), np.float32)
    small1[:N, 0:4] = x[:, None] * s1[None, :]
    small1[:N, 8] = x
    ad1_full = np.zeros((NV, 4), np.float32)
    ad1_full[:N] = x[:, None] * d1[None, :]

    w1rows = np.zeros((H, D), np.float32)
    for h in range(H):
        w1rows[h, h * HID:(h + 1) * HID] = w1col[h * HID:(h + 1) * HID]

    W2T = np.ascontiguousarray(W2.T)
    As2 = np.zeros((D, H), np.float32)
    Ad2 = np.zeros((D, H), np.float32)
    for h in range(H):
        As2[h * HID:(h + 1) * HID, h] = as2[h]
        Ad2[h * HID:(h + 1) * HID, h] = ad2[h]
    Bs2 = W2T @ As2
    Bd2 = W2T @ Ad2
    w2pre = np.ascontiguousarray(W2T.reshape(4, P, D).transpose(1, 0, 2))
    b2c = np.ascontiguousarray(
        np.concatenate([Bs2, Bd2], 1).reshape(4, P, 8).transpose(1, 0, 2))

    W3T = np.ascontiguousarray(W3.T)
    Bs3 = W3T @ as3.T
    Bd3 = W3T @ ad3.T
    w3pre = np.ascontiguousarray(W3T.reshape(4, P, HID).transpose(1, 0, 2))
    b3c = np.ascontiguousarray(
        np.concatenate([Bs3, Bd3], 1).reshape(4, P, 2).transpose(1, 0, 2))

    gbe1 = np.concatenate([g1.reshape(4, P).T, be1.reshape(4, P).T], 1)
    gbe2 = np.concatenate([g2.reshape(4, P).T, be2.reshape(4, P).T], 1)
    b3bc = np.tile(b3[None, :], (P, 1)).astype(np.float32)

    cnt = np.bincount(batch, minlength=G).astype(np.float32)
    rcnt = (1.0 / np.maximum(cnt, 1.0)).reshape(G, 1).astype(np.float32)

    ptile_all, mask_all = [], []
    for c in range(NCORES):
        pt = np.zeros((P, T, G), np.float32)
        mk = np.zeros((P, T), np.float32)
        base = c * pernode
        for t in range(T):
            ids = base + t * P + np.arange(P)
            real = ids < N
            mk[real, t] = 1.0
            bb = batch[ids[real]]
            pt[np.arange(P)[real], t, bb] = 1.0
        ptile_all.append(pt)
        mask_all.append(mk)

    iota = np.tile(np.arange(P, dtype=np.float32)[None, :], (P, 1))

    common = dict(
        small1=small1,
        iota=np.ascontiguousarray(iota, np.float32),
        w1rows=w1rows,
        w2pre=w2pre.astype(np.float32), b2c=b2c.astype(np.float32),
        w3pre=w3pre.astype(np.float32), b3c=b3c.astype(np.float32),
        gbe1=gbe1.astype(np.float32), gbe2=gbe2.astype(np.float32),
        b3bc=b3bc,
        rcnt=rcnt,
        lw1T=np.ascontiguousarray(lw1.T, np.float32),
        lb1r=lb1.reshape(1, -1).astype(np.float32),
        lw2T=np.ascontiguousarray(lw2.T, np.float32),
        lb2r=lb2.reshape(1, -1).astype(np.float32),
    )
    in_maps = []
    for c in range(NCORES):
        m = dict(common)
        m["idx_src"] = idx_src_all[c]
        m["ad1loc"] = np.ascontiguousarray(
            ad1_full[c * pernode:(c + 1) * pernode])
        m["dstloc"] = dstloc_all[c]
        m["ptile"] = ptile_all[c]
        m["maskt"] = mask_all[c]
        in_maps.append(m)
    return cfg, in_maps


# ----------------------------------------------------------------------------
# entry point
# ----------------------------------------------------------------------------

_CACHE = {}


def _get_program(cfg):
    key = cfg.key()
    if key not in _CACHE:
        nc = build_program(cfg)
        nc.compile()
        _CACHE[key] = nc
    return _CACHE[key]


def kernel(**inputs):
    cfg, in_maps = make_cfg_and_inputs(inputs)
    nc = _get_program(cfg)
    res = run_bass_kernel_spmd(nc, in_maps, core_ids=list(range(NCORES)))
    return np.asarray(res.results[0]["out"])
